# revision 1
# baseline (speedup 1.0000x reference)
"""Dehazing kernel for AWS Trainium2 (Bass/Tile), 8-core data-parallel.

Problem: img [32,3,512,512] f32, w [32] f32 ->
  dc  = 15x15 box-mean of per-pixel channel-min (zero-padded, /225)
  A_c = mean of img_c at the top-5% dc positions (k=13107 per image)
  t   = max(1 - w*dc, 0.1); out = clip((img-A)/(t+0.001) + A, 0, 1)

Sharding: pure data-parallel, batch 32 -> 8 NeuronCores x 4 images.
Per core, per image:
  - channel-min on DVE (2 tensor_tensor min ops)
  - horizontal 15-tap box sum: DVE prefix-scan + shifted subtract
    (+ small edge patches; zero-pad semantics match avg_pool2d
    count_include_pad)
  - vertical 15-tap box sum: PE banded-matrix matmuls (0/1 band
    matrices passed in as a constant input tensor); PSUM->SBUF copy
    applies the 1/225 scale on the Scalar engine
  - top-5% threshold: two-level per-partition "stripe grid" seed
    (one fused count pass per level) then 12 exact bisection rounds;
    each count is split DVE (lower half, is_ge+accum) / ACT (upper
    half, Sign+accum); cross-partition count reduction via a
    ones-matrix matmul on the (otherwise idle) tensor engine, which
    also broadcasts the total to all partitions
  - masked channel sums: fused scalar_tensor_tensor with accum_out
  - A = S/count (count==k except for sub-ULP ties, where the full tie
    set is averaged); dehaze: fused DVE ops, Relu(+A) on ACT,
    min-clamp on DVE, store in-place over the img tiles
"""
import os
import numpy as np

import concourse.bacc as bacc
import concourse.tile as tile
import concourse.mybir as mybir
from concourse.bass_utils import run_bass_kernel_spmd

F32 = mybir.dt.float32
I32 = mybir.dt.int32
U32 = mybir.dt.uint32
ALU = mybir.AluOpType
ACTF = mybir.ActivationFunctionType

P = 128
H = W = 512
G = H // P              # 4 row-groups
NPC = 4                 # images per core
K = 13107               # int(512*512*0.05)
KF = float(K)
NDVE_CNT = 896             # DVE count slice (per partition)
NACT_CNT = 2048 - NDVE_CNT  # ACT count slice (per partition)
NACT_TOT = float(NACT_CNT * P)
NTOT = float(H * W)
FULL_ROUNDS = 12

# grid-seed constants
D1 = 1.0 / 128.0            # level-1 grid step over [0,1)
L1_OFF = -3.0               # lo1 = (jcnt1 + L1_OFF) * D1
W1 = 5.0 * D1               # level-1 bracket width
D2 = W1 / 128.0             # level-2 grid step
L2_OFF = -4.0               # lo2 = lo1 + (jcnt2 + L2_OFF) * D2
W2 = 7.0 * D2               # bracket width entering full rounds
THR_DVE = KF / 128.0        # per-stripe count threshold (is_ge counts)
THR_ACT = 2.0 * KF / 128.0 - 2048.0  # same in sign-sum units


def make_consts() -> np.ndarray:
    k = np.arange(P)[:, None]
    m = np.arange(P)[None, :]
    bdiag = (np.abs(k - m) <= 7).astype(np.float32)
    bup = ((k - m) >= 121).astype(np.float32)
    bdn = ((m - k) >= 121).astype(np.float32)
    ones = np.ones((P, P), dtype=np.float32)
    return np.concatenate([bdiag, bup, bdn, ones], axis=1)  # [128, 512]


def build(nc):
    img_in = nc.dram_tensor("img", [NPC, 3, H, W], F32, kind="ExternalInput").ap()
    w_in = nc.dram_tensor("w", [NPC], F32, kind="ExternalInput").ap()
    consts_in = nc.dram_tensor("consts", [P, 4 * P], F32, kind="ExternalInput").ap()
    out_d = nc.dram_tensor("out", [NPC, 3, H, W], F32, kind="ExternalOutput").ap()

    with tile.TileContext(nc) as tc:
        with (
            tc.tile_pool(name="const", bufs=1) as const_pool,
            tc.tile_pool(name="img", bufs=4) as img_pool,
            tc.tile_pool(name="dcp", bufs=4) as dc_pool,
            tc.tile_pool(name="wk1p", bufs=2) as wk1p,
            tc.tile_pool(name="work", bufs=1) as work,
            tc.tile_pool(name="pbp", bufs=1) as pbp,
            tc.tile_pool(name="scnt", bufs=2) as scnt,
            tc.tile_pool(name="scnt2", bufs=2) as scnt2,
            tc.tile_pool(name="small", bufs=4) as small,
            tc.tile_pool(name="vband", bufs=2, space="PSUM") as vband,
            tc.tile_pool(name="cntps", bufs=2, space="PSUM") as cnt_ps,
            tc.tile_pool(name="miscps", bufs=1, space="PSUM") as misc_ps,
        ):
            consts = const_pool.tile([P, 4 * P], F32)
            nc.sync.dma_start(consts[:], consts_in[:])
            bdiag = consts[:, 0:P]
            bup = consts[:, P:2 * P]
            bdn = consts[:, 2 * P:3 * P]
            ones = consts[:, 3 * P:4 * P]

            # full-round combined compare: cnt_dve + 0.5*sum_act >= K - NHALF/2
            kvec_full = const_pool.tile([P, 2], F32)
            nc.vector.memset(kvec_full[:], KF - NACT_TOT / 2.0)

            # iota grid for the seed: g1[p] = p * D1 (and negated for ACT bias)
            grid_i = const_pool.tile([P, 1], I32)
            nc.gpsimd.iota(grid_i[:], pattern=[[0, 1]], base=0,
                           channel_multiplier=1)
            gridf = const_pool.tile([P, 1], F32)
            nc.vector.tensor_copy(gridf[:], grid_i[:])
            g1 = const_pool.tile([P, 1], F32)
            nc.vector.tensor_scalar(out=g1[:], in0=gridf[:], scalar1=D1,
                                    scalar2=None, op0=ALU.mult)
            ng1 = const_pool.tile([P, 1], F32)
            nc.vector.tensor_scalar(out=ng1[:], in0=g1[:], scalar1=-1.0,
                                    scalar2=None, op0=ALU.mult)

            w_sb = const_pool.tile([1, NPC], F32)
            nc.sync.dma_start(w_sb[:], w_in.rearrange("(p a) -> p a", p=1))
            w4_ps = misc_ps.tile([P, NPC], F32, tag="w4")
            nc.tensor.matmul(w4_ps[:], lhsT=ones[0:1, :], rhs=w_sb[:],
                             start=True, stop=True)
            negw4 = const_pool.tile([P, NPC], F32)
            nc.vector.tensor_scalar(out=negw4[:], in0=w4_ps[:], scalar1=-1.0,
                                    scalar2=None, op0=ALU.mult)

            def phase1(i):
                """load + channel-min + box filter -> (img tiles, dc tile)"""
                imgt = []
                for c in range(3):
                    t = img_pool.tile([P, G, W], F32, tag=f"img{c}")
                    nc.sync.dma_start(
                        t[:], img_in[i, c].rearrange("(g p) x -> p g x", p=P))
                    imgt.append(t)

                mn = wk1p.tile([P, G, W], F32, tag="wk1")
                nc.vector.tensor_tensor(out=mn[:], in0=imgt[0][:],
                                        in1=imgt[1][:], op=ALU.min)
                nc.vector.tensor_tensor(out=mn[:], in0=mn[:], in1=imgt[2][:],
                                        op=ALU.min)

                Pb = pbp.tile([P, 2056], F32, tag="pb")
                nc.vector.memset(Pb[:, 0:1], 0.0)
                mn_flat = mn[:].rearrange("p g x -> p (g x)")
                nc.vector.tensor_tensor_scan(
                    out=Pb[:, 1:2049], data0=mn_flat, data1=mn_flat,
                    initial=0.0, op0=ALU.add, op1=ALU.bypass)
                sh = mn  # sh overwrites mn's storage
                pv = Pb[:, 1:2049].rearrange("p (g x) -> p g x", g=G)
                nc.vector.tensor_tensor(
                    out=sh[:, :, 8:505], in0=pv[:, :, 15:512],
                    in1=pv[:, :, 0:497], op=ALU.subtract)
                for g in range(G):
                    base = g * W
                    nc.vector.tensor_tensor(
                        out=sh[:, g, 0:8], in0=Pb[:, base + 8:base + 16],
                        in1=Pb[:, base:base + 1].to_broadcast([P, 8]),
                        op=ALU.subtract)
                    nc.vector.tensor_tensor(
                        out=sh[:, g, 505:512],
                        in0=Pb[:, base + 512:base + 513].to_broadcast([P, 7]),
                        in1=Pb[:, base + 498:base + 505], op=ALU.subtract)

                dc = dc_pool.tile([P, G, W], F32, tag="dc")
                for gp in range(G):
                    ps = vband.tile([P, W], F32, tag="vps")
                    mms = [(bdiag, gp)]
                    if gp > 0:
                        mms.append((bup, gp - 1))
                    if gp < G - 1:
                        mms.append((bdn, gp + 1))
                    for j, (band, gsrc) in enumerate(mms):
                        nc.tensor.matmul(ps[:], lhsT=band, rhs=sh[:, gsrc, :],
                                         start=(j == 0), stop=(j == len(mms) - 1))
                    nc.scalar.activation(dc[:, gp, :], ps[:], ACTF.Copy,
                                         scale=1.0 / 225.0)
                return imgt, dc

            def grid_pass(i, dc_flat, thr_vec, out_col):
                """one stripe-grid counting pass + locate; writes jcnt into
                out_col [P,1] (broadcast). thr_vec: [P,1] thresholds."""
                cp = small.tile([P, 1], F32, tag="cp")
                scr = scnt.tile([P, 2 * W], F32, tag="scr")
                if i % 2 == 0:
                    nc.vector.tensor_scalar(
                        out=scr[:, :W * 2], in0=dc_flat[:, 0:2 * W],
                        scalar1=thr_vec, scalar2=None,
                        op0=ALU.is_ge, op1=ALU.add, accum_out=cp[:])
                    nc.vector.tensor_scalar(
                        out=scr[:, :W * 2], in0=dc_flat[:, 2 * W:4 * W],
                        scalar1=thr_vec, scalar2=None,
                        op0=ALU.is_ge, op1=ALU.add, accum_out=cp[:])
                    # NOTE: second accum overwrites; handled by caller variant
                return cp

            def seed_image(i, dc, lo4, wd4):
                """two-level stripe-grid seed for image i -> col of lo4/wd4
                (per-pair [P,2] state tiles, column i%2)."""
                dc_flat = dc[:].rearrange("p g x -> p (g x)")
                use_dve = (i % 2 == 0)
                # level 1
                cp = small.tile([P, 1], F32, tag="cp")
                scr = scnt.tile([P, G * W], F32, tag="scr")
                if use_dve:
                    nc.vector.tensor_scalar(
                        out=scr[:], in0=dc_flat, scalar1=g1[:], scalar2=None,
                        op0=ALU.is_ge, op1=ALU.add, accum_out=cp[:])
                    thr = THR_DVE
                else:
                    nc.scalar.activation(
                        scr[:], dc_flat, ACTF.Sign, bias=ng1[:], scale=1.0,
                        accum_out=cp[:])
                    thr = THR_ACT
                mk = small.tile([P, 1], F32, tag="mk")
                nc.vector.tensor_scalar(out=mk[:], in0=cp[:], scalar1=thr,
                                        scalar2=None, op0=ALU.is_ge)
                jc = cnt_ps.tile([P, 1], F32, tag="cps0")
                nc.tensor.matmul(jc[:], lhsT=ones, rhs=mk[:], start=True,
                                 stop=True)
                lo1 = small.tile([P, 1], F32, tag="lo1")
                nc.vector.tensor_scalar(out=lo1[:], in0=jc[:], scalar1=L1_OFF,
                                        scalar2=D1, op0=ALU.add, op1=ALU.mult)
                # level 2
                t2 = small.tile([P, 1], F32, tag="t2")
                nc.vector.scalar_tensor_tensor(
                    out=t2[:], in0=gridf[:], scalar=D2, in1=lo1[:],
                    op0=ALU.mult, op1=ALU.add)
                cp2 = small.tile([P, 1], F32, tag="cp")
                scr2 = scnt.tile([P, G * W], F32, tag="scr")
                if use_dve:
                    nc.vector.tensor_scalar(
                        out=scr2[:], in0=dc_flat, scalar1=t2[:], scalar2=None,
                        op0=ALU.is_ge, op1=ALU.add, accum_out=cp2[:])
                else:
                    nt2 = small.tile([P, 1], F32, tag="nt2")
                    nc.vector.tensor_scalar(out=nt2[:], in0=t2[:], scalar1=-1.0,
                                            scalar2=None, op0=ALU.mult)
                    nc.scalar.activation(
                        scr2[:], dc_flat, ACTF.Sign, bias=nt2[:], scale=1.0,
                        accum_out=cp2[:])
                mk2 = small.tile([P, 1], F32, tag="mk")
                nc.vector.tensor_scalar(out=mk2[:], in0=cp2[:], scalar1=thr,
                                        scalar2=None, op0=ALU.is_ge)
                jc2 = cnt_ps.tile([P, 1], F32, tag="cps0")
                nc.tensor.matmul(jc2[:], lhsT=ones, rhs=mk2[:], start=True,
                                 stop=True)
                q = small.tile([P, 1], F32, tag="q")
                nc.vector.tensor_scalar(out=q[:], in0=jc2[:], scalar1=L2_OFF,
                                        scalar2=D2, op0=ALU.add, op1=ALU.mult)
                j = i % 2
                nc.vector.tensor_tensor(out=lo4[:, j:j + 1], in0=q[:],
                                        in1=lo1[:], op=ALU.add)
                nc.vector.memset(wd4[:, j:j + 1], W2)

            def full_round(pair, dcs, lo2, wd2):
                """one bisection round for a pair; half-split DVE/ACT counts."""
                tau2 = small.tile([P, 2], F32, tag=f"tau{pair}")
                nc.vector.scalar_tensor_tensor(
                    out=tau2[:], in0=wd2[:], scalar=0.5, in1=lo2[:],
                    op0=ALU.mult, op1=ALU.add)
                ntau2 = small.tile([P, 2], F32, tag=f"ntau{pair}")
                nc.vector.tensor_scalar(
                    out=ntau2[:], in0=tau2[:], scalar1=-1.0,
                    scalar2=None, op0=ALU.mult)
                part4 = small.tile([P, 4], F32, tag=f"part{pair}")
                for j in range(2):
                    dflat = dcs[j][:].rearrange("p g x -> p (g x)")
                    scr = scnt2.tile([P, NACT_CNT], F32, tag="scr2")
                    nc.vector.tensor_scalar(
                        out=scr[:, :NDVE_CNT], in0=dflat[:, 0:NDVE_CNT],
                        scalar1=tau2[:, j:j + 1], scalar2=None,
                        op0=ALU.is_ge, op1=ALU.add,
                        accum_out=part4[:, 2 * j:2 * j + 1])
                    scr2 = scnt2.tile([P, NACT_CNT], F32, tag="scr2")
                    nc.scalar.activation(
                        scr2[:], dflat[:, NDVE_CNT:4 * W], ACTF.Sign,
                        bias=ntau2[:, j:j + 1], scale=1.0,
                        accum_out=part4[:, 2 * j + 1:2 * j + 2])
                cps = cnt_ps.tile([P, 4], F32, tag=f"cps{pair}")
                nc.tensor.matmul(cps[:], lhsT=ones, rhs=part4[:],
                                 start=True, stop=True)
                csb = small.tile([P, 4], F32, tag=f"csb{pair}")
                nc.scalar.activation(csb[:], cps[:], ACTF.Copy)
                cv = csb[:].rearrange("p (i s) -> p i s", s=2)
                u2 = small.tile([P, 2], F32, tag=f"u{pair}")
                nc.vector.scalar_tensor_tensor(
                    out=u2[:], in0=cv[:, :, 1], scalar=0.5, in1=cv[:, :, 0],
                    op0=ALU.mult, op1=ALU.add)
                a2 = small.tile([P, 2], U32, tag=f"cmp{pair}")
                nc.vector.tensor_tensor(out=a2[:], in0=u2[:], in1=kvec_full[:],
                                        op=ALU.is_ge)
                nc.vector.copy_predicated(lo2[:], a2[:], tau2[:])
                nc.vector.tensor_scalar(out=wd2[:], in0=wd2[:], scalar1=0.5,
                                        scalar2=None, op0=ALU.mult)

            def finals(i, imgt, dc, lo):
                dc_flat = dc[:].rearrange("p g x -> p (g x)")
                tm = work.tile([P, G * W], F32, tag="wk2")
                nc.vector.tensor_scalar(
                    out=tm[:], in0=dc_flat, scalar1=negw4[:, i:i + 1],
                    scalar2=1.0, op0=ALU.mult, op1=ALU.add)
                nc.vector.tensor_scalar(
                    out=tm[:], in0=tm[:], scalar1=0.001, scalar2=0.101,
                    op0=ALU.add, op1=ALU.max)
                rr = work.tile([P, G * W], F32, tag="wk3")
                nc.vector.reciprocal_approx_fast(out=rr[:], in_=tm[:])

                part4 = small.tile([P, 4], F32, tag="part4")
                nlo = small.tile([P, 1], F32, tag="nlo")
                nc.vector.tensor_scalar(out=nlo[:], in0=lo, scalar1=-1.0,
                                        scalar2=None, op0=ALU.mult)
                scrM = scnt.tile([P, G * W], F32, tag="scr")
                nc.scalar.activation(
                    scrM[:], dc_flat, ACTF.Sign, bias=nlo[:], scale=1.0,
                    accum_out=part4[:, 0:1])
                nc.vector.tensor_scalar(
                    out=part4[:, 0:1], in0=part4[:, 0:1], scalar1=2048.0,
                    scalar2=0.5, op0=ALU.add, op1=ALU.mult)
                for c in range(3):
                    scr_c = scnt.tile([P, G * W], F32, tag="scr")
                    nc.vector.scalar_tensor_tensor(
                        out=scr_c[:], in0=dc_flat, scalar=lo,
                        in1=imgt[c][:].rearrange("p g x -> p (g x)"),
                        op0=ALU.is_ge, op1=ALU.mult,
                        accum_out=part4[:, c + 1:c + 2])
                tot_ps = misc_ps.tile([P, 4], F32, tag="tot")
                nc.tensor.matmul(tot_ps[:], lhsT=ones, rhs=part4[:],
                                 start=True, stop=True)
                rcount = small.tile([P, 1], F32, tag="rcount")
                nc.vector.reciprocal(out=rcount[:], in_=tot_ps[:, 0:1])
                A3 = small.tile([P, 3], F32, tag="A3")
                nc.vector.tensor_tensor(out=A3[:], in0=tot_ps[:, 1:4],
                                        in1=rcount[:].to_broadcast([P, 3]),
                                        op=ALU.mult)

                for c in range(3):
                    img_flat = imgt[c][:].rearrange("p g x -> p (g x)")
                    d = work.tile([P, G * W], F32, tag="wk4")
                    nc.vector.scalar_tensor_tensor(
                        out=d[:], in0=img_flat, scalar=A3[:, c:c + 1], in1=rr[:],
                        op0=ALU.subtract, op1=ALU.mult)
                    nc.scalar.activation(d[:], d[:], ACTF.Relu,
                                         bias=A3[:, c:c + 1], scale=1.0)
                    nc.vector.tensor_scalar(out=img_flat, in0=d[:],
                                            scalar1=1.0, scalar2=None,
                                            op0=ALU.min)
                    nc.sync.dma_start(
                        out_d[i, c].rearrange("(g p) x -> p g x", p=P),
                        imgt[c][:])

            imgs, dcs = [], []
            for i in range(NPC):
                a, b = phase1(i)
                imgs.append(a)
                dcs.append(b)
            states = []
            for pair in range(2):
                lo2 = small.tile([P, 2], F32, tag=f"lo{pair}")
                wd2 = small.tile([P, 2], F32, tag=f"wd{pair}")
                states.append((lo2, wd2))
            for i in range(NPC):
                lo2, wd2 = states[i // 2]
                seed_image(i, dcs[i], lo2, wd2)
            for _ in range(FULL_ROUNDS):
                for pair in range(2):
                    lo2, wd2 = states[pair]
                    full_round(pair, dcs[2 * pair:2 * pair + 2], lo2, wd2)
            for i in range(NPC):
                lo2 = states[i // 2][0]
                finals(i, imgs[i], dcs[i], lo2[:, i % 2:i % 2 + 1])
    nc.compile()
    return nc


NCORES = 8
CONSTS = make_consts()
LAST_RESULT = None
_NC_CACHE = None


def _get_nc():
    global _NC_CACHE
    if _NC_CACHE is None:
        nc = bacc.Bacc("TRN2", target_bir_lowering=False, debug=False)
        _NC_CACHE = build(nc)
    return _NC_CACHE


def kernel(img: np.ndarray, w: np.ndarray) -> np.ndarray:
    global LAST_RESULT
    img = np.ascontiguousarray(np.asarray(img, dtype=np.float32))
    w = np.ascontiguousarray(np.asarray(w, dtype=np.float32))
    nc = _get_nc()
    in_maps = [
        {"img": img[i * NPC:(i + 1) * NPC], "w": w[i * NPC:(i + 1) * NPC],
         "consts": CONSTS}
        for i in range(NCORES)
    ]
    trace = bool(int(os.environ.get("DEHAZE_TRACE", "0")))
    res = run_bass_kernel_spmd(nc, in_maps, list(range(NCORES)), trace=trace)
    LAST_RESULT = res
    return np.concatenate([r["out"] for r in res.results], axis=0)



# revision 9
# speedup vs baseline: 1.3971x; 1.3971x over previous
"""Dehazing kernel for AWS Trainium2 (Bass/Tile), 8-core data-parallel.

Problem: img [32,3,512,512] f32, w [32] f32 ->
  dc  = 15x15 box-mean of per-pixel channel-min (zero-padded, /225)
  A_c = mean of img_c at the top-5% dc positions (k=13107 per image)
  t   = max(1 - w*dc, 0.1); out = clip((img-A)/(t+0.001) + A, 0, 1)

Sharding: pure data-parallel, batch 32 -> 8 NeuronCores x 4 images.

Per core, per image (Pool engine has no elementwise support in this
toolchain; work is split DVE / ACT / PE):
  - img channels cast to fp16 (ACT) so DVE elementwise runs in 2x mode
  - channel-min on DVE in fp16; horizontal 15-tap box sum: DVE fp32
    prefix-scan + shifted subtract, output fp16
  - vertical 15-tap box sum: PE banded-matrix matmuls in fp16 with the
    1/225 scale folded in as 1/256 (dc scaled by 225/256; w scaled by
    256/225 to compensate); PSUM->SBUF copies on ACT; dc kept fp32
  - top-5% threshold: 2-level stripe-grid seed (L1 half-sampled, L2
    full) bracketing the threshold, then 2 secant steps with global
    counts (c0/c2 half-sampled for the slope, c(tau1) full), all counts
    on ACT (Sign+accum); cross-partition reduction via ones matmul
  - finals: mask at tau2 on DVE (is_ge, fp16 out, accum gives the exact
    count), masked channel sums as fp16 stt (mask*img) with accum;
    A = sums/count; transmission: ACT Identity (scale=-w', bias=1.001)
    + DVE fast reciprocal + fp16 min-clamp; dehaze in fp16 on DVE with
    the upper clip as min(d, 1-A) and lower clip via +A / max0 (DVE) or
    Relu (ACT); output stored fp16 and upcast to fp32 on host
"""
import os
import numpy as np

import concourse.bacc as bacc
import concourse.tile as tile
import concourse.mybir as mybir
from concourse.bass_utils import run_bass_kernel_spmd

F32 = mybir.dt.float32
F16 = mybir.dt.float16
ALU = mybir.AluOpType
ACTF = mybir.ActivationFunctionType

P = 128
H = W = 512
G = H // P              # 4 row-groups
NPC = 4                 # images per core
K = 13107               # int(512*512*0.05)
KF = float(K)
KH = KF / 2.0           # target in half-sample count units
NTOT = float(H * W)

# stripe-grid seed constants (dc' = dc*225/256 lives in [0,1))
D1 = 1.0 / 128.0            # level-1 grid step
L1_OFF = -3.0               # lo1 = (jc1 + L1_OFF) * D1
D2 = 5.0 * D1 / 128.0       # level-2 grid step
L2_OFF = -10.0              # lo2 = lo1 + (jc2 + L2_OFF) * D2
W2 = 17.0 * D2              # bracket width for the secant stage
THR1_ACT = KF / 128.0 - 1024.0   # L1 stripe thr in Sign-sum units (half)
THR2_ACT = 2.0 * KF / 128.0 - 2048.0  # L2 stripe thr (full)
RRMAX = float(np.float32(1.0) / np.float32(0.101))
WSCALE = 256.0 / 225.0


def make_consts16() -> np.ndarray:
    k = np.arange(P)[:, None]
    m = np.arange(P)[None, :]
    v = np.float16(1.0 / 256.0)
    bdiag = (np.abs(k - m) <= 7).astype(np.float16) * v
    bup = ((k - m) >= 121).astype(np.float16) * v
    bdn = ((m - k) >= 121).astype(np.float16) * v
    return np.concatenate([bdiag, bup, bdn], axis=1)  # [128, 384] f16


def make_consts32() -> np.ndarray:
    return np.ones((P, P), dtype=np.float32)


def build(nc):
    img_in = nc.dram_tensor("img", [NPC, 3, H, W], F32, kind="ExternalInput").ap()
    w_in = nc.dram_tensor("w", [NPC], F32, kind="ExternalInput").ap()
    c16_in = nc.dram_tensor("c16", [P, 3 * P], F16, kind="ExternalInput").ap()
    c32_in = nc.dram_tensor("c32", [P, P], F32, kind="ExternalInput").ap()
    out_d = nc.dram_tensor("out", [NPC, 3, H, W], F16, kind="ExternalOutput").ap()

    V = nc.vector
    S = nc.scalar

    with tile.TileContext(nc) as tc:
        with (
            tc.tile_pool(name="const", bufs=1) as const_pool,
            tc.tile_pool(name="img32", bufs=2) as img32p,
            tc.tile_pool(name="img16", bufs=4) as img16p,
            tc.tile_pool(name="dcp", bufs=4) as dc_pool,
            tc.tile_pool(name="mnp", bufs=1) as mnp,
            tc.tile_pool(name="shp", bufs=2) as shp,
            tc.tile_pool(name="pbp", bufs=1) as pbp,
            tc.tile_pool(name="s16", bufs=1) as s16,
            tc.tile_pool(name="msk", bufs=2) as mskp,
            tc.tile_pool(name="rrp", bufs=2) as rrp,
            tc.tile_pool(name="d16", bufs=4) as d16p,
            tc.tile_pool(name="tm32", bufs=1) as tm32p,
            tc.tile_pool(name="small", bufs=4) as small,
            tc.tile_pool(name="vband", bufs=2, space="PSUM") as vband,
            tc.tile_pool(name="cntps", bufs=2, space="PSUM") as cnt_ps,
            tc.tile_pool(name="totps", bufs=2, space="PSUM") as tot_psp,
            tc.tile_pool(name="miscps", bufs=1, space="PSUM") as misc_ps,
        ):
            c16 = const_pool.tile([P, 3 * P], F16)
            nc.sync.dma_start(c16[:], c16_in[:])
            bdiag = c16[:, 0:P]
            bup = c16[:, P:2 * P]
            bdn = c16[:, 2 * P:3 * P]
            ones = const_pool.tile([P, P], F32)
            nc.sync.dma_start(ones[:], c32_in[:])

            # iota grid: gridf[p] = p (f32) and ng1[p] = -p*D1
            grid_i = const_pool.tile([P, 1], mybir.dt.int32)
            nc.gpsimd.iota(grid_i[:], pattern=[[0, 1]], base=0,
                           channel_multiplier=1)
            gridf = const_pool.tile([P, 1], F32)
            V.tensor_copy(gridf[:], grid_i[:])
            ng1 = const_pool.tile([P, 1], F32)
            V.tensor_scalar(out=ng1[:], in0=gridf[:], scalar1=-D1,
                            scalar2=None, op0=ALU.mult)

            # w broadcast to all partitions, scaled by -256/225
            w_sb = const_pool.tile([1, NPC], F32)
            nc.sync.dma_start(w_sb[:], w_in.rearrange("(p a) -> p a", p=1))
            w4_ps = misc_ps.tile([P, NPC], F32, tag="w4")
            nc.tensor.matmul(w4_ps[:], lhsT=ones[0:1, :], rhs=w_sb[:],
                             start=True, stop=True)
            negw4 = const_pool.tile([P, NPC], F32)
            V.tensor_scalar(out=negw4[:], in0=w4_ps[:], scalar1=-WSCALE,
                            scalar2=None, op0=ALU.mult)
            b1001 = const_pool.tile([P, 1], F32)
            V.memset(b1001[:], 1.001)

            def phase1(i):
                """load + fp16 cast + channel-min + box filter."""
                im16 = []
                for c in range(3):
                    t32 = img32p.tile([P, G, W], F32, tag=f"i32_{c}")
                    nc.sync.dma_start(
                        t32[:], img_in[i, c].rearrange("(g p) x -> p g x", p=P))
                    t16 = img16p.tile([P, G, W], F16, tag=f"i16_{c}")
                    S.activation(t16[:], t32[:], ACTF.Copy)
                    im16.append(t16)

                mn = mnp.tile([P, G, W], F16, tag="mn")
                V.tensor_tensor(out=mn[:], in0=im16[0][:], in1=im16[1][:],
                                op=ALU.min)
                V.tensor_tensor(out=mn[:], in0=mn[:], in1=im16[2][:],
                                op=ALU.min)

                Pb = pbp.tile([P, 2560], F32, tag="pb")
                V.memset(Pb[:, 0:1], 0.0)
                mn_flat = mn[:].rearrange("p g x -> p (g x)")
                V.tensor_tensor_scan(
                    out=Pb[:, 1:2049], data0=mn_flat, data1=mn_flat,
                    initial=0.0, op0=ALU.add, op1=ALU.bypass)
                sh = shp.tile([P, G, W], F16, tag="sh")
                pv = Pb[:, 1:2049].rearrange("p (g x) -> p g x", g=G)
                V.tensor_tensor(
                    out=sh[:, :, 8:505], in0=pv[:, :, 15:512],
                    in1=pv[:, :, 0:497], op=ALU.subtract)
                # group-strided views of Pb for the left/right edge columns
                pvL = Pb[:, 8:2056].rearrange("p (g x) -> p g x", x=W)
                pvB = Pb[:, 0:2048].rearrange("p (g x) -> p g x", x=W)
                V.tensor_tensor(
                    out=sh[:, :, 0:8], in0=pvL[:, :, 0:8],
                    in1=pvB[:, :, 0:1].to_broadcast([P, G, 8]),
                    op=ALU.subtract)
                pvR = Pb[:, 512:2560].rearrange("p (g x) -> p g x", x=W)
                pvS = Pb[:, 498:2546].rearrange("p (g x) -> p g x", x=W)
                V.tensor_tensor(
                    out=sh[:, :, 505:512],
                    in0=pvR[:, :, 0:1].to_broadcast([P, G, 7]),
                    in1=pvS[:, :, 0:7], op=ALU.subtract)

                dc = dc_pool.tile([P, G, W], F32, tag="dc")
                for gp in range(G):
                    ps = vband.tile([P, W], F32, tag="vps")
                    mms = [(bdiag, gp)]
                    if gp > 0:
                        mms.append((bup, gp - 1))
                    if gp < G - 1:
                        mms.append((bdn, gp + 1))
                    for j, (band, gsrc) in enumerate(mms):
                        nc.tensor.matmul(ps[:], lhsT=band, rhs=sh[:, gsrc, :],
                                         start=(j == 0), stop=(j == len(mms) - 1))
                    S.copy(dc[:, gp, :], ps[:])
                return im16, dc

            def halfview(dcf):
                return dcf.rearrange("p (n two) -> p n two", two=2)[:, :, 0]

            def count_matmul(part, n):
                """sum part [P,n] across partitions -> PSUM [P,n] bcast."""
                cps = cnt_ps.tile([P, n], F32, tag="cps")
                nc.tensor.matmul(cps[:], lhsT=ones, rhs=part[:, 0:n],
                                 start=True, stop=True)
                return cps

            def seed1(i, dc, st):
                """L1 stripe pass (ACT, half-sampled) -> lo1 tile."""
                dcf = dc[:].rearrange("p g x -> p (g x)")
                scr = s16.tile([P, 2048], F16, tag="sA")
                s1 = small.tile([P, 1], F32, tag="s1")
                S.activation(scr[:, 0:1024], halfview(dcf), ACTF.Sign,
                             bias=ng1[:], scale=1.0, accum_out=s1[:])
                mk = small.tile([P, 1], F32, tag="mk")
                V.tensor_scalar(out=mk[:], in0=s1[:], scalar1=THR1_ACT,
                                scalar2=None, op0=ALU.is_ge)
                jc = count_matmul(mk, 1)
                lo1 = small.tile([P, 1], F32, tag=f"lo1_{i}")
                V.tensor_scalar(out=lo1[:], in0=jc[:], scalar1=L1_OFF,
                                scalar2=D1, op0=ALU.add, op1=ALU.mult)
                st["lo1"] = lo1

            def seed2(i, dc, st):
                """L2 stripe pass (ACT, full) -> lo2, t2v (bracket)."""
                dcf = dc[:].rearrange("p g x -> p (g x)")
                lo1 = st["lo1"]
                nlo1 = small.tile([P, 1], F32, tag="nlo1")
                V.tensor_scalar(out=nlo1[:], in0=lo1[:], scalar1=-1.0,
                                scalar2=None, op0=ALU.mult)
                nt2g = small.tile([P, 1], F32, tag="nt2g")
                V.scalar_tensor_tensor(
                    out=nt2g[:], in0=gridf[:], scalar=-D2, in1=nlo1[:],
                    op0=ALU.mult, op1=ALU.add)
                scr = s16.tile([P, 2048], F16, tag="sA")
                s2 = small.tile([P, 1], F32, tag="s2")
                S.activation(scr[:], dcf, ACTF.Sign, bias=nt2g[:], scale=1.0,
                             accum_out=s2[:])
                mk = small.tile([P, 1], F32, tag="mk")
                V.tensor_scalar(out=mk[:], in0=s2[:], scalar1=THR2_ACT,
                                scalar2=None, op0=ALU.is_ge)
                jc2 = count_matmul(mk, 1)
                q = small.tile([P, 1], F32, tag="q")
                V.tensor_scalar(out=q[:], in0=jc2[:], scalar1=L2_OFF,
                                scalar2=D2, op0=ALU.add, op1=ALU.mult)
                lo2 = small.tile([P, 1], F32, tag=f"lo2_{i}")
                V.tensor_tensor(out=lo2[:], in0=q[:], in1=lo1[:], op=ALU.add)
                t2v = small.tile([P, 1], F32, tag=f"t2v_{i}")
                V.tensor_scalar(out=t2v[:], in0=lo2[:], scalar1=W2,
                                scalar2=None, op0=ALU.add)
                nlo2 = small.tile([P, 1], F32, tag=f"nlo2_{i}")
                V.tensor_scalar(out=nlo2[:], in0=lo2[:], scalar1=-1.0,
                                scalar2=None, op0=ALU.mult)
                st["lo2"], st["t2v"], st["nlo2"] = lo2, t2v, nlo2

            def slope(i, dc, st):
                """c0/c2 (ACT Sign, half-sampled) -> rW, tau1 seed."""
                dcf = dc[:].rearrange("p g x -> p (g x)")
                dch = halfview(dcf)
                lo2, t2v, nlo2 = st["lo2"], st["t2v"], st["nlo2"]
                part2 = small.tile([P, 2], F32, tag=f"pc_{i}")
                scra = s16.tile([P, 2048], F16, tag="sA")
                S.activation(scra[:, 0:1024], dch, ACTF.Sign, bias=nlo2[:],
                             scale=1.0, accum_out=part2[:, 0:1])
                nt2v = small.tile([P, 1], F32, tag="nt2v")
                V.tensor_scalar(out=nt2v[:], in0=t2v[:], scalar1=-1.0,
                                scalar2=None, op0=ALU.mult)
                scrb = s16.tile([P, 2048], F16, tag="sA")
                S.activation(scrb[:, 0:1024], dch, ACTF.Sign, bias=nt2v[:],
                             scale=1.0, accum_out=part2[:, 1:2])
                cps = count_matmul(part2, 2)
                # both slots: half-count = 0.5*S + 65536
                cv = small.tile([P, 2], F32, tag="cv")
                V.tensor_scalar(out=cv[:], in0=cps[:], scalar1=131072.0,
                                scalar2=0.5, op0=ALU.add, op1=ALU.mult)
                diff = small.tile([P, 1], F32, tag="diff")
                V.tensor_tensor(out=diff[:], in0=cv[:, 0:1], in1=cv[:, 1:2],
                                op=ALU.subtract)
                V.tensor_scalar(out=diff[:], in0=diff[:], scalar1=0.5,
                                scalar2=None, op0=ALU.max)
                rd = small.tile([P, 1], F32, tag="rd")
                V.reciprocal(out=rd[:], in_=diff[:])
                rW = small.tile([P, 1], F32, tag=f"rW_{i}")
                V.tensor_scalar(out=rW[:], in0=rd[:], scalar1=W2,
                                scalar2=None, op0=ALU.mult)
                e0 = small.tile([P, 1], F32, tag="e0")
                V.tensor_scalar(out=e0[:], in0=cv[:, 0:1], scalar1=-KH,
                                scalar2=None, op0=ALU.add)
                tau1 = small.tile([P, 1], F32, tag=f"tau1_{i}")
                V.scalar_tensor_tensor(
                    out=tau1[:], in0=e0[:], scalar=rW[:], in1=lo2[:],
                    op0=ALU.mult, op1=ALU.add)
                V.tensor_scalar(out=tau1[:], in0=tau1[:], scalar1=lo2[:],
                                scalar2=t2v[:], op0=ALU.max, op1=ALU.min)
                st["rW"], st["tau1"] = rW, tau1

            def refine(i, dc, st):
                """count at tau1 (ACT, full) -> tau2 (final threshold)."""
                dcf = dc[:].rearrange("p g x -> p (g x)")
                tau1, rW = st["tau1"], st["rW"]
                lo2, t2v = st["lo2"], st["t2v"]
                ntau1 = small.tile([P, 1], F32, tag="ntau1")
                V.tensor_scalar(out=ntau1[:], in0=tau1[:], scalar1=-1.0,
                                scalar2=None, op0=ALU.mult)
                scra = s16.tile([P, 2048], F16, tag="sA")
                pc1 = small.tile([P, 1], F32, tag="pc1")
                S.activation(scra[:], dcf, ACTF.Sign, bias=ntau1[:],
                             scale=1.0, accum_out=pc1[:])
                ct = count_matmul(pc1, 1)
                # half-count units: c1h = 0.25*S + 65536
                e1 = small.tile([P, 1], F32, tag="e1")
                V.tensor_scalar(out=e1[:], in0=ct[:], scalar1=262144.0,
                                scalar2=0.25, op0=ALU.add, op1=ALU.mult)
                V.tensor_scalar(out=e1[:], in0=e1[:], scalar1=-KH,
                                scalar2=None, op0=ALU.add)
                tau2 = small.tile([P, 1], F32, tag=f"tau2_{i}")
                V.scalar_tensor_tensor(
                    out=tau2[:], in0=e1[:], scalar=rW[:], in1=tau1[:],
                    op0=ALU.mult, op1=ALU.add)
                V.tensor_scalar(out=tau2[:], in0=tau2[:], scalar1=lo2[:],
                                scalar2=t2v[:], op0=ALU.max, op1=ALU.min)
                st["tau2"] = tau2

            def finals(i, im16, dc, st):
                dcf = dc[:].rearrange("p g x -> p (g x)")
                tau2 = st["tau2"]
                # transmission denominator: tm = 1.001 - w'*dc'  (ACT)
                tm = tm32p.tile([P, 2048], F32, tag="tm")
                S.activation(tm[:], dcf, ACTF.Identity, bias=b1001[:],
                             scale=negw4[:, i:i + 1])
                # mask at tau2 (fp16) + exact count via accum (DVE)
                part4 = small.tile([P, 4], F32, tag=f"p4_{i}")
                mask = mskp.tile([P, 2048], F16, tag="mask")
                V.tensor_scalar(out=mask[:], in0=dcf, scalar1=tau2[:],
                                scalar2=None, op0=ALU.is_ge, op1=ALU.add,
                                accum_out=part4[:, 0:1])
                # masked channel sums: (mask*1)*img, all-fp16 stt (DVE 2x)
                for c in range(3):
                    scr = s16.tile([P, 2048], F16, tag="sD")
                    V.scalar_tensor_tensor(
                        out=scr[:], in0=mask[:], scalar=1.0,
                        in1=im16[c][:].rearrange("p g x -> p (g x)"),
                        op0=ALU.mult, op1=ALU.mult,
                        accum_out=part4[:, c + 1:c + 2])
                tot = tot_psp.tile([P, 4], F32, tag="tot")
                nc.tensor.matmul(tot[:], lhsT=ones, rhs=part4[:],
                                 start=True, stop=True)
                rc = small.tile([P, 1], F32, tag="rc")
                V.reciprocal(out=rc[:], in_=tot[:, 0:1])
                A3 = small.tile([P, 3], F32, tag=f"A3_{i}")
                V.tensor_tensor(out=A3[:], in0=tot[:, 1:4],
                                in1=rc[:].to_broadcast([P, 3]), op=ALU.mult)
                omA3 = small.tile([P, 3], F32, tag=f"om_{i}")
                V.tensor_scalar(out=omA3[:], in0=A3[:], scalar1=-1.0,
                                scalar2=1.0, op0=ALU.mult, op1=ALU.add)
                # rr = min(1/tm, 1/0.101) in fp16; 1/tm overwrites dc
                V.reciprocal_approx_fast(out=dcf, in_=tm[:])
                rr = rrp.tile([P, 2048], F16, tag="rr")
                V.tensor_scalar(out=rr[:], in0=dcf, scalar1=RRMAX,
                                scalar2=None, op0=ALU.min)
                # dehaze: d=(img-A)*rr; out=clip(d+A,0,1)
                #   upper clip first: min(d, 1-A); lower clip via +A, max 0
                for c in range(3):
                    img_flat = im16[c][:].rearrange("p g x -> p (g x)")
                    d = d16p.tile([P, 2048], F16, tag="d")
                    V.scalar_tensor_tensor(
                        out=d[:], in0=img_flat, scalar=A3[:, c:c + 1],
                        in1=rr[:], op0=ALU.subtract, op1=ALU.mult)
                    V.tensor_scalar(out=d[:], in0=d[:],
                                    scalar1=omA3[:, c:c + 1],
                                    scalar2=None, op0=ALU.min)
                    if c == 0:
                        V.tensor_scalar(out=d[:], in0=d[:],
                                        scalar1=A3[:, c:c + 1], scalar2=0.0,
                                        op0=ALU.add, op1=ALU.max)
                    else:
                        S.activation(d[:], d[:], ACTF.Relu,
                                     bias=A3[:, c:c + 1], scale=1.0)
                    nc.sync.dma_start(
                        out_d[i, c].rearrange("(g p) x -> p g x", p=P),
                        d[:])

            imgs, dcs = [], []
            for i in range(NPC):
                a, b = phase1(i)
                imgs.append(a)
                dcs.append(b)
            states = [dict() for _ in range(NPC)]
            for i in range(NPC):
                seed1(i, dcs[i], states[i])
            for i in range(NPC):
                seed2(i, dcs[i], states[i])
            for i in range(NPC):
                slope(i, dcs[i], states[i])
            for i in range(NPC):
                refine(i, dcs[i], states[i])
            for i in range(NPC):
                finals(i, imgs[i], dcs[i], states[i])
    nc.compile()
    return nc


NCORES = 8
CONSTS16 = make_consts16()
CONSTS32 = make_consts32()
LAST_RESULT = None
_NC_CACHE = None


def _get_nc():
    global _NC_CACHE
    if _NC_CACHE is None:
        nc = bacc.Bacc("TRN2", target_bir_lowering=False, debug=False)
        _NC_CACHE = build(nc)
    return _NC_CACHE


def kernel(img: np.ndarray, w: np.ndarray) -> np.ndarray:
    global LAST_RESULT
    img = np.ascontiguousarray(np.asarray(img, dtype=np.float32))
    w = np.ascontiguousarray(np.asarray(w, dtype=np.float32))
    nc = _get_nc()
    in_maps = [
        {"img": img[i * NPC:(i + 1) * NPC], "w": w[i * NPC:(i + 1) * NPC],
         "c16": CONSTS16, "c32": CONSTS32}
        for i in range(NCORES)
    ]
    trace = bool(int(os.environ.get("DEHAZE_TRACE", "0")))
    res = run_bass_kernel_spmd(nc, in_maps, list(range(NCORES)), trace=trace)
    LAST_RESULT = res
    return np.concatenate(
        [r["out"] for r in res.results], axis=0).astype(np.float32)


# revision 10
# speedup vs baseline: 1.4336x; 1.0262x over previous
"""Dehazing kernel for AWS Trainium2 (Bass/Tile), 8-core data-parallel.

Problem: img [32,3,512,512] f32, w [32] f32 ->
  dc  = 15x15 box-mean of per-pixel channel-min (zero-padded, /225)
  A_c = mean of img_c at the top-5% dc positions (k=13107 per image)
  t   = max(1 - w*dc, 0.1); out = clip((img-A)/(t+0.001) + A, 0, 1)

Sharding: pure data-parallel, batch 32 -> 8 NeuronCores x 4 images.

Per core, per image (work split DVE / ACT / PE):
  - one batched DMA load per image; channels cast to fp16 in one ACT op
    so DVE elementwise runs in its 2x packed mode where supported
  - channel-min on DVE fp16 (2x); horizontal 15-tap box sum: DVE fp32
    prefix-scan + shifted subtract (fp16 out, edge columns as two
    group-strided ops)
  - vertical 15-tap box sum: PE banded-matrix matmuls in fp16 with the
    1/225 scale folded in as 1/256 (dc scaled by 225/256; w scaled by
    256/225 to compensate); all 4 row-groups accumulate into one
    [P,2048] PSUM tile copied to SBUF in a single ACT op
  - top-5% threshold: 2-level stripe-grid seed (L1 half-sampled, L2
    full) bracketing the threshold, then 2 secant steps with global
    counts (c0/c2 half-sampled slope, c(tau1) full). All counts on ACT
    as Sign(tau - dc) (scale=-1, bias=tau: no negation smalls);
    cross-partition reduction + broadcast via ones matmul
  - finals: exact recount at tau2 on ACT; masked channel sums as DVE
    stt (is_ge * img) with accum; A = sums/count; transmission: ACT
    Identity (scale=-w', bias=1.001) + DVE fast reciprocal + fp16
    min-clamp; dehaze d=(img-A)*rr on DVE, clip via min/addmax on DVE
    (ch0) or ACT Relu pairs out = Relu(1 - Relu((1-A) - d)) (ch1/ch2);
    one batched fp16 store per image, upcast to fp32 on host
"""
import os
import numpy as np

import concourse.bacc as bacc
import concourse.tile as tile
import concourse.mybir as mybir
from concourse.bass_utils import run_bass_kernel_spmd

F32 = mybir.dt.float32
F16 = mybir.dt.float16
ALU = mybir.AluOpType
ACTF = mybir.ActivationFunctionType

P = 128
H = W = 512
G = H // P              # 4 row-groups
NPC = 4                 # images per core
K = 13107               # int(512*512*0.05)
KF = float(K)
KH = KF / 2.0           # target in half-sample count units
NTOT = float(H * W)

# stripe-grid seed constants (dc' = dc*225/256 lives in [0,1))
D1 = 1.0 / 128.0            # level-1 grid step
L1_OFF = -3.0               # lo1 = (jc1 + L1_OFF) * D1
D2 = 5.0 * D1 / 128.0       # level-2 grid step
L2_OFF = -10.0              # lo2 = lo1 + (jc2 + L2_OFF) * D2
W2 = 17.0 * D2              # bracket width for the secant stage
# counts use S' = sum sign(tau - x); count_ge = (N - S')/2
THR1N = 1024.0 - KF / 128.0      # L1 stripe: S'_p <= THR1N
THR2N = 2048.0 - 2.0 * KF / 128.0  # L2 stripe
RRMAX = float(np.float32(1.0) / np.float32(0.101))
WSCALE = 256.0 / 225.0


def make_consts16() -> np.ndarray:
    k = np.arange(P)[:, None]
    m = np.arange(P)[None, :]
    v = np.float16(1.0 / 256.0)
    bdiag = (np.abs(k - m) <= 7).astype(np.float16) * v
    bup = ((k - m) >= 121).astype(np.float16) * v
    bdn = ((m - k) >= 121).astype(np.float16) * v
    return np.concatenate([bdiag, bup, bdn], axis=1)  # [128, 384] f16


def make_consts32() -> np.ndarray:
    return np.ones((P, P), dtype=np.float32)


def build(nc):
    img_in = nc.dram_tensor("img", [NPC, 3, H, W], F32, kind="ExternalInput").ap()
    w_in = nc.dram_tensor("w", [NPC], F32, kind="ExternalInput").ap()
    c16_in = nc.dram_tensor("c16", [P, 3 * P], F16, kind="ExternalInput").ap()
    c32_in = nc.dram_tensor("c32", [P, P], F32, kind="ExternalInput").ap()
    out_d = nc.dram_tensor("out", [NPC, 3, H, W], F16, kind="ExternalOutput").ap()

    V = nc.vector
    S = nc.scalar

    with tile.TileContext(nc) as tc:
        with (
            tc.tile_pool(name="const", bufs=1) as const_pool,
            tc.tile_pool(name="img32", bufs=2) as img32p,
            tc.tile_pool(name="img16", bufs=4) as img16p,
            tc.tile_pool(name="dcp", bufs=4) as dc_pool,
            tc.tile_pool(name="mnp", bufs=1) as mnp,
            tc.tile_pool(name="shp", bufs=2) as shp,
            tc.tile_pool(name="pbp", bufs=1) as pbp,
            tc.tile_pool(name="s16", bufs=1) as s16,
            tc.tile_pool(name="rrp", bufs=2) as rrp,
            tc.tile_pool(name="dp", bufs=2) as dpp,
            tc.tile_pool(name="tm32", bufs=1) as tm32p,
            tc.tile_pool(name="small", bufs=4) as small,
            tc.tile_pool(name="vband", bufs=1, space="PSUM") as vband,
            tc.tile_pool(name="cntps", bufs=2, space="PSUM") as cnt_ps,
            tc.tile_pool(name="totps", bufs=1, space="PSUM") as tot_psp,
            tc.tile_pool(name="miscps", bufs=1, space="PSUM") as misc_ps,
        ):
            c16 = const_pool.tile([P, 3 * P], F16)
            nc.sync.dma_start(c16[:], c16_in[:])
            bdiag = c16[:, 0:P]
            bup = c16[:, P:2 * P]
            bdn = c16[:, 2 * P:3 * P]
            ones = const_pool.tile([P, P], F32)
            nc.sync.dma_start(ones[:], c32_in[:])

            # iota grid: gridf[p] = p and g1[p] = p*D1
            grid_i = const_pool.tile([P, 1], mybir.dt.int32)
            nc.gpsimd.iota(grid_i[:], pattern=[[0, 1]], base=0,
                           channel_multiplier=1)
            gridf = const_pool.tile([P, 1], F32)
            V.tensor_copy(gridf[:], grid_i[:])
            g1 = const_pool.tile([P, 1], F32)
            V.tensor_scalar(out=g1[:], in0=gridf[:], scalar1=D1,
                            scalar2=None, op0=ALU.mult)

            # w broadcast to all partitions, scaled by -256/225
            w_sb = const_pool.tile([1, NPC], F32)
            nc.sync.dma_start(w_sb[:], w_in.rearrange("(p a) -> p a", p=1))
            w4_ps = misc_ps.tile([P, NPC], F32, tag="w4")
            nc.tensor.matmul(w4_ps[:], lhsT=ones[0:1, :], rhs=w_sb[:],
                             start=True, stop=True)
            negw4 = const_pool.tile([P, NPC], F32)
            V.tensor_scalar(out=negw4[:], in0=w4_ps[:], scalar1=-WSCALE,
                            scalar2=None, op0=ALU.mult)
            b1001 = const_pool.tile([P, 1], F32)
            V.memset(b1001[:], 1.001)
            bone = const_pool.tile([P, 1], F32)
            V.memset(bone[:], 1.0)

            def phase1(i):
                """batched load + fp16 cast + channel-min + box filter."""
                t32 = img32p.tile([P, 3, G, W], F32, tag="i32")
                nc.sync.dma_start(
                    t32[:], img_in[i].rearrange("c (g p) x -> p c g x", p=P))
                t16 = img16p.tile([P, 3, G, W], F16, tag="i16")
                S.activation(t16[:].rearrange("p c g x -> p (c g x)"),
                             t32[:].rearrange("p c g x -> p (c g x)"),
                             ACTF.Copy)

                mn = mnp.tile([P, G, W], F16, tag="mn")
                V.tensor_tensor(out=mn[:], in0=t16[:, 0], in1=t16[:, 1],
                                op=ALU.min)
                V.tensor_tensor(out=mn[:], in0=mn[:], in1=t16[:, 2],
                                op=ALU.min)

                Pb = pbp.tile([P, 2560], F32, tag="pb")
                V.memset(Pb[:, 0:1], 0.0)
                mn_flat = mn[:].rearrange("p g x -> p (g x)")
                V.tensor_tensor_scan(
                    out=Pb[:, 1:2049], data0=mn_flat, data1=mn_flat,
                    initial=0.0, op0=ALU.add, op1=ALU.bypass)
                sh = shp.tile([P, G, W], F16, tag="sh")
                pv = Pb[:, 1:2049].rearrange("p (g x) -> p g x", g=G)
                V.tensor_tensor(
                    out=sh[:, :, 8:505], in0=pv[:, :, 15:512],
                    in1=pv[:, :, 0:497], op=ALU.subtract)
                # group-strided views of Pb for the left/right edge columns
                pvL = Pb[:, 8:2056].rearrange("p (g x) -> p g x", x=W)
                pvB = Pb[:, 0:2048].rearrange("p (g x) -> p g x", x=W)
                V.tensor_tensor(
                    out=sh[:, :, 0:8], in0=pvL[:, :, 0:8],
                    in1=pvB[:, :, 0:1].to_broadcast([P, G, 8]),
                    op=ALU.subtract)
                pvR = Pb[:, 512:2560].rearrange("p (g x) -> p g x", x=W)
                pvS = Pb[:, 498:2546].rearrange("p (g x) -> p g x", x=W)
                V.tensor_tensor(
                    out=sh[:, :, 505:512],
                    in0=pvR[:, :, 0:1].to_broadcast([P, G, 7]),
                    in1=pvS[:, :, 0:7], op=ALU.subtract)

                # vertical band matmuls, all groups into one PSUM tile
                ps = vband.tile([P, G, W], F32, tag="vps")
                for gp in range(G):
                    mms = [(bdiag, gp)]
                    if gp > 0:
                        mms.append((bup, gp - 1))
                    if gp < G - 1:
                        mms.append((bdn, gp + 1))
                    for j, (band, gsrc) in enumerate(mms):
                        nc.tensor.matmul(ps[:, gp], lhsT=band,
                                         rhs=sh[:, gsrc, :],
                                         start=(j == 0), stop=(j == len(mms) - 1))
                dc = dc_pool.tile([P, G, W], F32, tag="dc")
                S.copy(dc[:].rearrange("p g x -> p (g x)"),
                       ps[:].rearrange("p g x -> p (g x)"))
                return t16, dc

            def halfview(dcf):
                return dcf.rearrange("p (n two) -> p n two", two=2)[:, :, 0]

            def count_matmul(part, n):
                """sum part [P,n] across partitions -> PSUM [P,n] bcast."""
                cps = cnt_ps.tile([P, n], F32, tag="cps")
                nc.tensor.matmul(cps[:], lhsT=ones, rhs=part[:, 0:n],
                                 start=True, stop=True)
                return cps

            def seed1(i, dc, st):
                """L1 stripe pass (ACT, half-sampled) -> lo1 tile."""
                dcf = dc[:].rearrange("p g x -> p (g x)")
                scr = s16.tile([P, 2048], F16, tag="sA")
                s1 = small.tile([P, 1], F32, tag="s1")
                S.activation(scr[:, 0:1024], halfview(dcf), ACTF.Sign,
                             bias=g1[:], scale=-1.0, accum_out=s1[:])
                mk = small.tile([P, 1], F32, tag="mk")
                V.tensor_scalar(out=mk[:], in0=s1[:], scalar1=THR1N,
                                scalar2=None, op0=ALU.is_le)
                jc = count_matmul(mk, 1)
                lo1 = small.tile([P, 1], F32, tag=f"lo1_{i}")
                V.tensor_scalar(out=lo1[:], in0=jc[:], scalar1=L1_OFF,
                                scalar2=D1, op0=ALU.add, op1=ALU.mult)
                st["lo1"] = lo1

            def seed2(i, dc, st):
                """L2 stripe pass (ACT, full) -> lo2, t2v (bracket)."""
                dcf = dc[:].rearrange("p g x -> p (g x)")
                lo1 = st["lo1"]
                t2g = small.tile([P, 1], F32, tag="t2g")
                V.scalar_tensor_tensor(
                    out=t2g[:], in0=gridf[:], scalar=D2, in1=lo1[:],
                    op0=ALU.mult, op1=ALU.add)
                scr = s16.tile([P, 2048], F16, tag="sA")
                s2 = small.tile([P, 1], F32, tag="s2")
                S.activation(scr[:], dcf, ACTF.Sign, bias=t2g[:], scale=-1.0,
                             accum_out=s2[:])
                mk = small.tile([P, 1], F32, tag="mk")
                V.tensor_scalar(out=mk[:], in0=s2[:], scalar1=THR2N,
                                scalar2=None, op0=ALU.is_le)
                jc2 = count_matmul(mk, 1)
                q = small.tile([P, 1], F32, tag="q")
                V.tensor_scalar(out=q[:], in0=jc2[:], scalar1=L2_OFF,
                                scalar2=D2, op0=ALU.add, op1=ALU.mult)
                lo2 = small.tile([P, 1], F32, tag=f"lo2_{i}")
                V.tensor_tensor(out=lo2[:], in0=q[:], in1=lo1[:], op=ALU.add)
                t2v = small.tile([P, 1], F32, tag=f"t2v_{i}")
                V.tensor_scalar(out=t2v[:], in0=lo2[:], scalar1=W2,
                                scalar2=None, op0=ALU.add)
                st["lo2"], st["t2v"] = lo2, t2v

            def slope(i, dc, st):
                """c0/c2 (ACT, half-sampled) -> rW, tau1 seed."""
                dcf = dc[:].rearrange("p g x -> p (g x)")
                dch = halfview(dcf)
                lo2, t2v = st["lo2"], st["t2v"]
                part2 = small.tile([P, 2], F32, tag=f"pc_{i}")
                scra = s16.tile([P, 2048], F16, tag="sA")
                S.activation(scra[:, 0:1024], dch, ACTF.Sign, bias=lo2[:],
                             scale=-1.0, accum_out=part2[:, 0:1])
                scrb = s16.tile([P, 2048], F16, tag="sA")
                S.activation(scrb[:, 0:1024], dch, ACTF.Sign, bias=t2v[:],
                             scale=-1.0, accum_out=part2[:, 1:2])
                cps = count_matmul(part2, 2)
                # half-count = -0.5*S' + 65536 for both slots
                cv = small.tile([P, 2], F32, tag="cv")
                V.tensor_scalar(out=cv[:], in0=cps[:], scalar1=-0.5,
                                scalar2=65536.0, op0=ALU.mult, op1=ALU.add)
                diff = small.tile([P, 1], F32, tag="diff")
                V.tensor_tensor(out=diff[:], in0=cv[:, 0:1], in1=cv[:, 1:2],
                                op=ALU.subtract)
                V.tensor_scalar(out=diff[:], in0=diff[:], scalar1=0.5,
                                scalar2=None, op0=ALU.max)
                rd = small.tile([P, 1], F32, tag="rd")
                V.reciprocal(out=rd[:], in_=diff[:])
                rW = small.tile([P, 1], F32, tag=f"rW_{i}")
                V.tensor_scalar(out=rW[:], in0=rd[:], scalar1=W2,
                                scalar2=None, op0=ALU.mult)
                e0 = small.tile([P, 1], F32, tag="e0")
                V.tensor_scalar(out=e0[:], in0=cv[:, 0:1], scalar1=-KH,
                                scalar2=None, op0=ALU.add)
                tau1 = small.tile([P, 1], F32, tag=f"tau1_{i}")
                V.scalar_tensor_tensor(
                    out=tau1[:], in0=e0[:], scalar=rW[:], in1=lo2[:],
                    op0=ALU.mult, op1=ALU.add)
                V.tensor_scalar(out=tau1[:], in0=tau1[:], scalar1=lo2[:],
                                scalar2=t2v[:], op0=ALU.max, op1=ALU.min)
                st["rW"], st["tau1"] = rW, tau1

            def refine(i, dc, st):
                """count at tau1 (ACT, full) -> tau2 (final threshold)."""
                dcf = dc[:].rearrange("p g x -> p (g x)")
                tau1, rW = st["tau1"], st["rW"]
                lo2, t2v = st["lo2"], st["t2v"]
                scra = s16.tile([P, 2048], F16, tag="sA")
                pc1 = small.tile([P, 1], F32, tag="pc1")
                S.activation(scra[:], dcf, ACTF.Sign, bias=tau1[:],
                             scale=-1.0, accum_out=pc1[:])
                ct = count_matmul(pc1, 1)
                # half-count units: c1h - KH = -0.25*S' + (65536 - KH)
                e1 = small.tile([P, 1], F32, tag="e1")
                V.tensor_scalar(out=e1[:], in0=ct[:], scalar1=-0.25,
                                scalar2=65536.0 - KH, op0=ALU.mult,
                                op1=ALU.add)
                tau2 = small.tile([P, 1], F32, tag=f"tau2_{i}")
                V.scalar_tensor_tensor(
                    out=tau2[:], in0=e1[:], scalar=rW[:], in1=tau1[:],
                    op0=ALU.mult, op1=ALU.add)
                V.tensor_scalar(out=tau2[:], in0=tau2[:], scalar1=lo2[:],
                                scalar2=t2v[:], op0=ALU.max, op1=ALU.min)
                st["tau2"] = tau2

            def finals_a(i, t16, dc, st):
                """recount + masked sums -> A3, omA3; tm (ACT)."""
                dcf = dc[:].rearrange("p g x -> p (g x)")
                tau2 = st["tau2"]
                tm = tm32p.tile([P, 2048], F32, tag="tm")
                S.activation(tm[:], dcf, ACTF.Identity, bias=b1001[:],
                             scale=negw4[:, i:i + 1])
                part4 = small.tile([P, 4], F32, tag=f"p4_{i}")
                scra = s16.tile([P, 2048], F16, tag="sA")
                S.activation(scra[:], dcf, ACTF.Sign, bias=tau2[:],
                             scale=-1.0, accum_out=part4[:, 0:1])
                for c in range(3):
                    scr = s16.tile([P, 2048], F16, tag="sD")
                    V.scalar_tensor_tensor(
                        out=scr[:], in0=dcf, scalar=tau2[:],
                        in1=t16[:, c].rearrange("p g x -> p (g x)"),
                        op0=ALU.is_ge, op1=ALU.mult,
                        accum_out=part4[:, c + 1:c + 2])
                tot = tot_psp.tile([P, 4], F32, tag="tot")
                nc.tensor.matmul(tot[:], lhsT=ones, rhs=part4[:],
                                 start=True, stop=True)
                cnt = small.tile([P, 1], F32, tag="cnt")
                V.tensor_scalar(out=cnt[:], in0=tot[:, 0:1], scalar1=-0.5,
                                scalar2=131072.0, op0=ALU.mult, op1=ALU.add)
                rc = small.tile([P, 1], F32, tag="rc")
                V.reciprocal(out=rc[:], in_=cnt[:])
                A3 = small.tile([P, 3], F32, tag=f"A3_{i}")
                V.tensor_tensor(out=A3[:], in0=tot[:, 1:4],
                                in1=rc[:].to_broadcast([P, 3]), op=ALU.mult)
                omA3 = small.tile([P, 3], F32, tag=f"om_{i}")
                V.tensor_scalar(out=omA3[:], in0=A3[:], scalar1=-1.0,
                                scalar2=1.0, op0=ALU.mult, op1=ALU.add)
                st["A3"], st["omA3"], st["tm"] = A3, omA3, tm

            def finals_b(i, t16, dc, st):
                """rr + dehaze + batched store."""
                dcf = dc[:].rearrange("p g x -> p (g x)")
                A3, omA3, tm = st["A3"], st["omA3"], st["tm"]
                # rr = min(1/tm, 1/0.101) fp16; 1/tm overwrites dc
                V.reciprocal_approx_fast(out=dcf, in_=tm[:])
                rr = rrp.tile([P, 2048], F16, tag="rr")
                V.tensor_scalar(out=rr[:], in0=dcf, scalar1=RRMAX,
                                scalar2=None, op0=ALU.min)
                d = dpp.tile([P, 3, G, W], F16, tag="d")
                for c in range(3):
                    img_flat = t16[:, c].rearrange("p g x -> p (g x)")
                    dv = d[:, c].rearrange("p g x -> p (g x)")
                    V.scalar_tensor_tensor(
                        out=dv, in0=img_flat, scalar=A3[:, c:c + 1],
                        in1=rr[:], op0=ALU.subtract, op1=ALU.mult)
                    if c == 0:
                        V.tensor_scalar(out=dv, in0=dv,
                                        scalar1=omA3[:, c:c + 1],
                                        scalar2=None, op0=ALU.min)
                        V.tensor_scalar(out=dv, in0=dv,
                                        scalar1=A3[:, c:c + 1], scalar2=0.0,
                                        op0=ALU.add, op1=ALU.max)
                    else:
                        # out = Relu(1 - Relu((1-A) - d)): both clips on ACT
                        u = s16.tile([P, 2048], F16, tag="sD")
                        S.activation(u[:], dv, ACTF.Relu,
                                     bias=omA3[:, c:c + 1], scale=-1.0)
                        S.activation(dv, u[:], ACTF.Relu,
                                     bias=bone[:], scale=-1.0)
                nc.sync.dma_start(
                    out_d[i].rearrange("c (g p) x -> p c g x", p=P), d[:])

            imgs, dcs = [], []
            for i in range(NPC):
                a, b = phase1(i)
                imgs.append(a)
                dcs.append(b)
            states = [dict() for _ in range(NPC)]
            for i in range(NPC):
                seed1(i, dcs[i], states[i])
            for i in range(NPC):
                seed2(i, dcs[i], states[i])
            for i in range(NPC):
                slope(i, dcs[i], states[i])
            for i in range(NPC):
                refine(i, dcs[i], states[i])
            for i in range(NPC):
                finals_a(i, imgs[i], dcs[i], states[i])
            for i in range(NPC):
                finals_b(i, imgs[i], dcs[i], states[i])
    nc.compile()
    return nc


NCORES = 8
CONSTS16 = make_consts16()
CONSTS32 = make_consts32()
LAST_RESULT = None
_NC_CACHE = None


def _get_nc():
    global _NC_CACHE
    if _NC_CACHE is None:
        nc = bacc.Bacc("TRN2", target_bir_lowering=False, debug=False)
        _NC_CACHE = build(nc)
    return _NC_CACHE


def kernel(img: np.ndarray, w: np.ndarray) -> np.ndarray:
    global LAST_RESULT
    img = np.ascontiguousarray(np.asarray(img, dtype=np.float32))
    w = np.ascontiguousarray(np.asarray(w, dtype=np.float32))
    nc = _get_nc()
    in_maps = [
        {"img": img[i * NPC:(i + 1) * NPC], "w": w[i * NPC:(i + 1) * NPC],
         "c16": CONSTS16, "c32": CONSTS32}
        for i in range(NCORES)
    ]
    trace = bool(int(os.environ.get("DEHAZE_TRACE", "0")))
    res = run_bass_kernel_spmd(nc, in_maps, list(range(NCORES)), trace=trace)
    LAST_RESULT = res
    return np.concatenate(
        [r["out"] for r in res.results], axis=0).astype(np.float32)


# revision 16
# speedup vs baseline: 1.4573x; 1.0165x over previous
"""Dehazing kernel for AWS Trainium2 (Bass/Tile), 8-core data-parallel.

Problem: img [32,3,512,512] f32, w [32] f32 ->
  dc  = 15x15 box-mean of per-pixel channel-min (zero-padded, /225)
  A_c = mean of img_c at the top-5% dc positions (k=13107 per image)
  t   = max(1 - w*dc, 0.1); out = clip((img-A)/(t+0.001) + A, 0, 1)

Sharding: pure data-parallel, batch 32 -> 8 NeuronCores x 4 images.

Per core, per image (work split DVE / ACT / PE):
  - one batched DMA load per image; channels cast to fp16 in one ACT op
    so DVE elementwise runs in its 2x packed mode where supported
  - channel-min on DVE fp16 (2x); horizontal 15-tap box sum: DVE fp32
    prefix-scan + shifted subtract (fp16 out, edge columns as two
    group-strided ops)
  - vertical 15-tap box sum: PE banded-matrix matmuls in fp16 with the
    1/225 scale folded in as 1/256 (dc scaled by 225/256; w scaled by
    256/225 to compensate); all 4 row-groups accumulate into one
    [P,2048] PSUM tile copied to SBUF in a single ACT op
  - top-5% threshold: 2-level stripe-grid seed (L1 half-sampled, L2
    full) bracketing the threshold, then 2 secant steps with global
    counts (c0/c2 half-sampled slope, c(tau1) full). All counts on ACT
    as Sign(tau - dc) (scale=-1, bias=tau: no negation smalls);
    cross-partition reduction + broadcast via ones matmul
  - finals: exact recount at tau2 on ACT; masked channel sums as DVE
    stt (is_ge * img) with accum; A = sums/count; transmission: ACT
    Identity (scale=-w', bias=1.001) + DVE fast reciprocal + fp16
    min-clamp; dehaze d=(img-A)*rr on DVE, clip via min/addmax on DVE
    (ch0) or ACT Relu pairs out = Relu(1 - Relu((1-A) - d)) (ch1/ch2);
    one batched fp16 store per image, upcast to fp32 on host
"""
import os
import numpy as np

import concourse.bacc as bacc
import concourse.tile as tile
import concourse.mybir as mybir
from concourse.bass_utils import run_bass_kernel_spmd

F32 = mybir.dt.float32
F16 = mybir.dt.float16
ALU = mybir.AluOpType
ACTF = mybir.ActivationFunctionType

P = 128
H = W = 512
G = H // P              # 4 row-groups
NPC = 4                 # images per core
K = 13107               # int(512*512*0.05)
KF = float(K)
KH = KF / 2.0           # target in half-sample count units
NTOT = float(H * W)

# stripe-grid seed constants (dc' = dc*225/256 lives in [0,1))
D1 = 1.0 / 128.0            # level-1 grid step
L1_OFF = -3.0               # lo1 = (jc1 + L1_OFF) * D1
D2 = 5.0 * D1 / 128.0       # level-2 grid step
L2_OFF = -10.0              # lo2 = lo1 + (jc2 + L2_OFF) * D2
W2 = 17.0 * D2              # bracket width for the secant stage
# counts use S' = sum sign(tau - x); count_ge = (N - S')/2
THR1N = 1024.0 - KF / 128.0      # L1 stripe: S'_p <= THR1N
THR2N = 2048.0 - 2.0 * KF / 128.0  # L2 stripe
RRMAX = float(np.float32(1.0) / np.float32(0.101))
WSCALE = 256.0 / 225.0


def make_consts16() -> np.ndarray:
    k = np.arange(P)[:, None]
    m = np.arange(P)[None, :]
    v = np.float16(1.0 / 256.0)
    bdiag = (np.abs(k - m) <= 7).astype(np.float16) * v
    bup = ((k - m) >= 121).astype(np.float16) * v
    bdn = ((m - k) >= 121).astype(np.float16) * v
    return np.concatenate([bdiag, bup, bdn], axis=1)  # [128, 384] f16


def make_consts32() -> np.ndarray:
    return np.ones((P, P), dtype=np.float32)


def build(nc):
    img_in = nc.dram_tensor("img", [NPC, 3, H, W], F32, kind="ExternalInput").ap()
    w_in = nc.dram_tensor("w", [NPC], F32, kind="ExternalInput").ap()
    c16_in = nc.dram_tensor("c16", [P, 3 * P], F16, kind="ExternalInput").ap()
    c32_in = nc.dram_tensor("c32", [P, P], F32, kind="ExternalInput").ap()
    out_d = nc.dram_tensor("out", [NPC, 3, H, W], F16, kind="ExternalOutput").ap()

    V = nc.vector
    S = nc.scalar

    with tile.TileContext(nc) as tc:
        with (
            tc.tile_pool(name="const", bufs=1) as const_pool,
            tc.tile_pool(name="img32", bufs=2) as img32p,
            tc.tile_pool(name="img16", bufs=4) as img16p,
            tc.tile_pool(name="dcp", bufs=4) as dc_pool,
            tc.tile_pool(name="mnp", bufs=1) as mnp,
            tc.tile_pool(name="shp", bufs=2) as shp,
            tc.tile_pool(name="pbp", bufs=1) as pbp,
            tc.tile_pool(name="s16", bufs=1) as s16,
            tc.tile_pool(name="rrp", bufs=2) as rrp,
            tc.tile_pool(name="dp", bufs=2) as dpp,
            tc.tile_pool(name="tm32", bufs=1) as tm32p,
            tc.tile_pool(name="small", bufs=4) as small,
            tc.tile_pool(name="vband", bufs=1, space="PSUM") as vband,
            tc.tile_pool(name="cntps", bufs=2, space="PSUM") as cnt_ps,
            tc.tile_pool(name="totps", bufs=1, space="PSUM") as tot_psp,
            tc.tile_pool(name="miscps", bufs=1, space="PSUM") as misc_ps,
        ):
            c16 = const_pool.tile([P, 3 * P], F16)
            nc.sync.dma_start(c16[:], c16_in[:])
            bdiag = c16[:, 0:P]
            bup = c16[:, P:2 * P]
            bdn = c16[:, 2 * P:3 * P]
            ones = const_pool.tile([P, P], F32)
            nc.sync.dma_start(ones[:], c32_in[:])

            # iota grid: gridf[p] = p and g1[p] = p*D1
            grid_i = const_pool.tile([P, 1], mybir.dt.int32)
            nc.gpsimd.iota(grid_i[:], pattern=[[0, 1]], base=0,
                           channel_multiplier=1)
            gridf = const_pool.tile([P, 1], F32)
            V.tensor_copy(gridf[:], grid_i[:])
            g1 = const_pool.tile([P, 1], F32)
            V.tensor_scalar(out=g1[:], in0=gridf[:], scalar1=D1,
                            scalar2=None, op0=ALU.mult)

            # w broadcast to all partitions, scaled by -256/225
            w_sb = const_pool.tile([1, NPC], F32)
            nc.sync.dma_start(w_sb[:], w_in.rearrange("(p a) -> p a", p=1))
            w4_ps = misc_ps.tile([P, NPC], F32, tag="w4")
            nc.tensor.matmul(w4_ps[:], lhsT=ones[0:1, :], rhs=w_sb[:],
                             start=True, stop=True)
            negw4 = const_pool.tile([P, NPC], F32)
            V.tensor_scalar(out=negw4[:], in0=w4_ps[:], scalar1=-WSCALE,
                            scalar2=None, op0=ALU.mult)
            b1001 = const_pool.tile([P, 1], F32)
            V.memset(b1001[:], 1.001)
            bone = const_pool.tile([P, 1], F32)
            V.memset(bone[:], 1.0)
            bthr1 = const_pool.tile([P, 1], F32)
            V.memset(bthr1[:], THR1N)
            bthr2 = const_pool.tile([P, 1], F32)
            V.memset(bthr2[:], THR2N)

            def phase1(i):
                """batched load + fp16 cast + channel-min + box filter."""
                t32 = img32p.tile([P, 3, G, W], F32, tag="i32")
                t16 = img16p.tile([P, 3, G, W], F16, tag="i16")
                for c in range(3):
                    nc.sync.dma_start(
                        t32[:, c],
                        img_in[i, c].rearrange("(g p) x -> p g x", p=P))
                    S.activation(t16[:, c].rearrange("p g x -> p (g x)"),
                                 t32[:, c].rearrange("p g x -> p (g x)"),
                                 ACTF.Copy)

                mn = mnp.tile([P, G, W], F16, tag="mn")
                V.tensor_tensor(out=mn[:], in0=t16[:, 0], in1=t16[:, 1],
                                op=ALU.min)
                V.tensor_tensor(out=mn[:], in0=mn[:], in1=t16[:, 2],
                                op=ALU.min)

                Pb = pbp.tile([P, 2560], F32, tag="pb")
                V.memset(Pb[:, 0:1], 0.0)
                mn_flat = mn[:].rearrange("p g x -> p (g x)")
                V.tensor_tensor_scan(
                    out=Pb[:, 1:2049], data0=mn_flat, data1=mn_flat,
                    initial=0.0, op0=ALU.add, op1=ALU.bypass)
                sh = shp.tile([P, G, W], F16, tag="sh")
                pv = Pb[:, 1:2049].rearrange("p (g x) -> p g x", g=G)
                V.tensor_tensor(
                    out=sh[:, :, 8:505], in0=pv[:, :, 15:512],
                    in1=pv[:, :, 0:497], op=ALU.subtract)
                # group-strided views of Pb for the left/right edge columns
                pvL = Pb[:, 8:2056].rearrange("p (g x) -> p g x", x=W)
                pvB = Pb[:, 0:2048].rearrange("p (g x) -> p g x", x=W)
                V.tensor_tensor(
                    out=sh[:, :, 0:8], in0=pvL[:, :, 0:8],
                    in1=pvB[:, :, 0:1].to_broadcast([P, G, 8]),
                    op=ALU.subtract)
                pvR = Pb[:, 512:2560].rearrange("p (g x) -> p g x", x=W)
                pvS = Pb[:, 498:2546].rearrange("p (g x) -> p g x", x=W)
                V.tensor_tensor(
                    out=sh[:, :, 505:512],
                    in0=pvR[:, :, 0:1].to_broadcast([P, G, 7]),
                    in1=pvS[:, :, 0:7], op=ALU.subtract)

                # vertical band matmuls, all groups into one PSUM tile
                ps = vband.tile([P, G, W], F32, tag="vps")
                for gp in range(G):
                    mms = [(bdiag, gp)]
                    if gp > 0:
                        mms.append((bup, gp - 1))
                    if gp < G - 1:
                        mms.append((bdn, gp + 1))
                    for j, (band, gsrc) in enumerate(mms):
                        nc.tensor.matmul(ps[:, gp], lhsT=band,
                                         rhs=sh[:, gsrc, :],
                                         start=(j == 0), stop=(j == len(mms) - 1))
                dc = dc_pool.tile([P, G, W], F32, tag="dc")
                S.copy(dc[:].rearrange("p g x -> p (g x)"),
                       ps[:].rearrange("p g x -> p (g x)"))
                return t16, dc

            def halfview(dcf):
                return dcf.rearrange("p (n two) -> p n two", two=2)[:, :, 0]

            def count_matmul(part, n):
                """sum part [P,n] across partitions -> PSUM [P,n] bcast."""
                cps = cnt_ps.tile([P, n], F32, tag="cps")
                nc.tensor.matmul(cps[:], lhsT=ones, rhs=part[:, 0:n],
                                 start=True, stop=True)
                return cps

            def seed1(i, dc, st):
                """L1 stripe pass (ACT, half-sampled) -> lo1 tile."""
                dcf = dc[:].rearrange("p g x -> p (g x)")
                scr = s16.tile([P, 2048], F16, tag="sA")
                s1 = small.tile([P, 1], F32, tag="s1")
                S.activation(scr[:, 0:1024], halfview(dcf), ACTF.Sign,
                             bias=g1[:], scale=-1.0, accum_out=s1[:])
                # mk = sign(THR1N - S') on ACT; jc = (sum(mk)+128)/2
                mk = small.tile([P, 1], F32, tag="mk")
                S.activation(mk[:], s1[:], ACTF.Sign, bias=bthr1[:],
                             scale=-1.0)
                jc = count_matmul(mk, 1)
                lo1 = small.tile([P, 1], F32, tag=f"lo1_{i}")
                V.tensor_scalar(out=lo1[:], in0=jc[:], scalar1=0.5 * D1,
                                scalar2=(64.0 + L1_OFF) * D1, op0=ALU.mult,
                                op1=ALU.add)
                st["lo1"] = lo1

            def seed2(i, dc, st):
                """L2 stripe pass (ACT, full) -> lo2, t2v (bracket)."""
                dcf = dc[:].rearrange("p g x -> p (g x)")
                lo1 = st["lo1"]
                t2g = small.tile([P, 1], F32, tag="t2g")
                V.scalar_tensor_tensor(
                    out=t2g[:], in0=gridf[:], scalar=D2, in1=lo1[:],
                    op0=ALU.mult, op1=ALU.add)
                scr = s16.tile([P, 2048], F16, tag="sA")
                s2 = small.tile([P, 1], F32, tag="s2")
                S.activation(scr[:], dcf, ACTF.Sign, bias=t2g[:], scale=-1.0,
                             accum_out=s2[:])
                mk = small.tile([P, 1], F32, tag="mk")
                S.activation(mk[:], s2[:], ACTF.Sign, bias=bthr2[:],
                             scale=-1.0)
                jc2 = count_matmul(mk, 1)
                q = small.tile([P, 1], F32, tag="q")
                V.tensor_scalar(out=q[:], in0=jc2[:], scalar1=0.5 * D2,
                                scalar2=(64.0 + L2_OFF) * D2, op0=ALU.mult,
                                op1=ALU.add)
                lo2 = small.tile([P, 1], F32, tag=f"lo2_{i}")
                V.tensor_tensor(out=lo2[:], in0=q[:], in1=lo1[:], op=ALU.add)
                t2v = small.tile([P, 1], F32, tag=f"t2v_{i}")
                V.tensor_scalar(out=t2v[:], in0=lo2[:], scalar1=W2,
                                scalar2=None, op0=ALU.add)
                st["lo2"], st["t2v"] = lo2, t2v

            def slope(i, dc, st):
                """c0/c2 (ACT, half-sampled) -> rW, tau1 seed."""
                dcf = dc[:].rearrange("p g x -> p (g x)")
                dch = halfview(dcf)
                lo2, t2v = st["lo2"], st["t2v"]
                part2 = small.tile([P, 2], F32, tag=f"pc_{i}")
                scra = s16.tile([P, 2048], F16, tag="sA")
                S.activation(scra[:, 0:1024], dch, ACTF.Sign, bias=lo2[:],
                             scale=-1.0, accum_out=part2[:, 0:1])
                scrb = s16.tile([P, 2048], F16, tag="sA")
                S.activation(scrb[:, 0:1024], dch, ACTF.Sign, bias=t2v[:],
                             scale=-1.0, accum_out=part2[:, 1:2])
                cps = count_matmul(part2, 2)
                # half-count = -0.5*S' + 65536 for both slots
                cv = small.tile([P, 2], F32, tag="cv")
                V.tensor_scalar(out=cv[:], in0=cps[:], scalar1=-0.5,
                                scalar2=65536.0, op0=ALU.mult, op1=ALU.add)
                diff = small.tile([P, 1], F32, tag="diff")
                V.tensor_tensor(out=diff[:], in0=cv[:, 0:1], in1=cv[:, 1:2],
                                op=ALU.subtract)
                V.tensor_scalar(out=diff[:], in0=diff[:], scalar1=0.5,
                                scalar2=None, op0=ALU.max)
                rd = small.tile([P, 1], F32, tag="rd")
                V.reciprocal(out=rd[:], in_=diff[:])
                rW = small.tile([P, 1], F32, tag=f"rW_{i}")
                V.tensor_scalar(out=rW[:], in0=rd[:], scalar1=W2,
                                scalar2=None, op0=ALU.mult)
                e0 = small.tile([P, 1], F32, tag="e0")
                V.tensor_scalar(out=e0[:], in0=cv[:, 0:1], scalar1=-KH,
                                scalar2=None, op0=ALU.add)
                tau1 = small.tile([P, 1], F32, tag=f"tau1_{i}")
                V.scalar_tensor_tensor(
                    out=tau1[:], in0=e0[:], scalar=rW[:], in1=lo2[:],
                    op0=ALU.mult, op1=ALU.add)
                V.tensor_scalar(out=tau1[:], in0=tau1[:], scalar1=lo2[:],
                                scalar2=t2v[:], op0=ALU.max, op1=ALU.min)
                st["rW"], st["tau1"] = rW, tau1

            def refine(i, dc, st):
                """count at tau1 (ACT, full) -> tau2 (final threshold)."""
                dcf = dc[:].rearrange("p g x -> p (g x)")
                tau1, rW = st["tau1"], st["rW"]
                lo2, t2v = st["lo2"], st["t2v"]
                scra = s16.tile([P, 2048], F16, tag="sA")
                pc1 = small.tile([P, 1], F32, tag="pc1")
                S.activation(scra[:], dcf, ACTF.Sign, bias=tau1[:],
                             scale=-1.0, accum_out=pc1[:])
                ct = count_matmul(pc1, 1)
                # half-count units: c1h - KH = -0.25*S' + (65536 - KH)
                e1 = small.tile([P, 1], F32, tag="e1")
                V.tensor_scalar(out=e1[:], in0=ct[:], scalar1=-0.25,
                                scalar2=65536.0 - KH, op0=ALU.mult,
                                op1=ALU.add)
                tau2 = small.tile([P, 1], F32, tag=f"tau2_{i}")
                V.scalar_tensor_tensor(
                    out=tau2[:], in0=e1[:], scalar=rW[:], in1=tau1[:],
                    op0=ALU.mult, op1=ALU.add)
                V.tensor_scalar(out=tau2[:], in0=tau2[:], scalar1=lo2[:],
                                scalar2=t2v[:], op0=ALU.max, op1=ALU.min)
                st["tau2"] = tau2

            def finals_a(i, t16, dc, st):
                """recount + masked sums -> A3, omA3; tm (ACT)."""
                dcf = dc[:].rearrange("p g x -> p (g x)")
                tau2 = st["tau2"]
                tm = tm32p.tile([P, 2048], F32, tag="tm")
                S.activation(tm[:], dcf, ACTF.Identity, bias=b1001[:],
                             scale=negw4[:, i:i + 1])
                part4 = small.tile([P, 4], F32, tag=f"p4_{i}")
                scra = s16.tile([P, 2048], F16, tag="sA")
                S.activation(scra[:], dcf, ACTF.Sign, bias=tau2[:],
                             scale=-1.0, accum_out=part4[:, 0:1])
                for c in range(3):
                    scr = s16.tile([P, 2048], F16, tag="sD")
                    V.scalar_tensor_tensor(
                        out=scr[:], in0=dcf, scalar=tau2[:],
                        in1=t16[:, c].rearrange("p g x -> p (g x)"),
                        op0=ALU.is_ge, op1=ALU.mult,
                        accum_out=part4[:, c + 1:c + 2])
                tot = tot_psp.tile([P, 4], F32, tag="tot")
                nc.tensor.matmul(tot[:], lhsT=ones, rhs=part4[:],
                                 start=True, stop=True)
                cnt = small.tile([P, 1], F32, tag="cnt")
                V.tensor_scalar(out=cnt[:], in0=tot[:, 0:1], scalar1=-0.5,
                                scalar2=131072.0, op0=ALU.mult, op1=ALU.add)
                rc = small.tile([P, 1], F32, tag="rc")
                V.reciprocal(out=rc[:], in_=cnt[:])
                A3 = small.tile([P, 3], F32, tag=f"A3_{i}")
                V.tensor_tensor(out=A3[:], in0=tot[:, 1:4],
                                in1=rc[:].to_broadcast([P, 3]), op=ALU.mult)
                omA3 = small.tile([P, 3], F32, tag=f"om_{i}")
                V.tensor_scalar(out=omA3[:], in0=A3[:], scalar1=-1.0,
                                scalar2=1.0, op0=ALU.mult, op1=ALU.add)
                st["A3"], st["omA3"], st["tm"] = A3, omA3, tm

            def finals_b(i, t16, dc, st):
                """rr + dehaze + batched store."""
                dcf = dc[:].rearrange("p g x -> p (g x)")
                A3, omA3, tm = st["A3"], st["omA3"], st["tm"]
                # rr = min(1/tm, 1/0.101) fp16; 1/tm overwrites dc
                V.reciprocal_approx_fast(out=dcf, in_=tm[:])
                rr = rrp.tile([P, 2048], F16, tag="rr")
                V.tensor_scalar(out=rr[:], in0=dcf, scalar1=RRMAX,
                                scalar2=None, op0=ALU.min)
                d = dpp.tile([P, 3, G, W], F16, tag="d")
                for c in range(3):
                    img_flat = t16[:, c].rearrange("p g x -> p (g x)")
                    dv = d[:, c].rearrange("p g x -> p (g x)")
                    V.scalar_tensor_tensor(
                        out=dv, in0=img_flat, scalar=A3[:, c:c + 1],
                        in1=rr[:], op0=ALU.subtract, op1=ALU.mult)
                    if c == 0:
                        V.tensor_scalar(out=dv, in0=dv,
                                        scalar1=omA3[:, c:c + 1],
                                        scalar2=None, op0=ALU.min)
                        V.tensor_scalar(out=dv, in0=dv,
                                        scalar1=A3[:, c:c + 1], scalar2=0.0,
                                        op0=ALU.add, op1=ALU.max)
                    else:
                        # out = Relu(1 - Relu((1-A) - d)): both clips on ACT
                        u = s16.tile([P, 2048], F16, tag="sD")
                        S.activation(u[:], dv, ACTF.Relu,
                                     bias=omA3[:, c:c + 1], scale=-1.0)
                        S.activation(dv, u[:], ACTF.Relu,
                                     bias=bone[:], scale=-1.0)
                    nc.sync.dma_start(
                        out_d[i, c].rearrange("(g p) x -> p g x", p=P),
                        d[:, c])

            imgs, dcs = [], []
            for i in range(NPC):
                a, b = phase1(i)
                imgs.append(a)
                dcs.append(b)
            states = [dict() for _ in range(NPC)]
            for i in range(NPC):
                seed1(i, dcs[i], states[i])
            for i in range(NPC):
                seed2(i, dcs[i], states[i])
            for i in range(NPC):
                slope(i, dcs[i], states[i])
            for i in range(NPC):
                refine(i, dcs[i], states[i])
            # interleave: ACT-heavy finals_a(i+1) overlaps DVE-heavy
            # finals_b(i)
            finals_a(0, imgs[0], dcs[0], states[0])
            finals_a(1, imgs[1], dcs[1], states[1])
            finals_b(0, imgs[0], dcs[0], states[0])
            finals_a(2, imgs[2], dcs[2], states[2])
            finals_b(1, imgs[1], dcs[1], states[1])
            finals_a(3, imgs[3], dcs[3], states[3])
            finals_b(2, imgs[2], dcs[2], states[2])
            finals_b(3, imgs[3], dcs[3], states[3])
    nc.compile()
    return nc


NCORES = 8
CONSTS16 = make_consts16()
CONSTS32 = make_consts32()
LAST_RESULT = None
_NC_CACHE = None


def _get_nc():
    global _NC_CACHE
    if _NC_CACHE is None:
        nc = bacc.Bacc("TRN2", target_bir_lowering=False, debug=False)
        _NC_CACHE = build(nc)
    return _NC_CACHE


def kernel(img: np.ndarray, w: np.ndarray) -> np.ndarray:
    global LAST_RESULT
    img = np.ascontiguousarray(np.asarray(img, dtype=np.float32))
    w = np.ascontiguousarray(np.asarray(w, dtype=np.float32))
    nc = _get_nc()
    in_maps = [
        {"img": img[i * NPC:(i + 1) * NPC], "w": w[i * NPC:(i + 1) * NPC],
         "c16": CONSTS16, "c32": CONSTS32}
        for i in range(NCORES)
    ]
    trace = bool(int(os.environ.get("DEHAZE_TRACE", "0")))
    res = run_bass_kernel_spmd(nc, in_maps, list(range(NCORES)), trace=trace)
    LAST_RESULT = res
    return np.concatenate(
        [r["out"] for r in res.results], axis=0).astype(np.float32)


# revision 19
# speedup vs baseline: 1.5473x; 1.0618x over previous
"""Dehazing kernel for AWS Trainium2 (Bass/Tile), 8-core data-parallel.

Problem: img [32,3,512,512] f32, w [32] f32 ->
  dc  = 15x15 box-mean of per-pixel channel-min (zero-padded, /225)
  A_c = mean of img_c at the top-5% dc positions (k=13107 per image)
  t   = max(1 - w*dc, 0.1); out = clip((img-A)/(t+0.001) + A, 0, 1)

Sharding: pure data-parallel, batch 32 -> 8 NeuronCores x 4 images.

Per core, per image (work split DVE / ACT / PE):
  - one batched DMA load per image; channels cast to fp16 in one ACT op
    so DVE elementwise runs in its 2x packed mode where supported
  - channel-min on DVE fp16 (2x); horizontal 15-tap box sum: DVE fp32
    prefix-scan + shifted subtract (fp16 out, edge columns as two
    group-strided ops)
  - vertical 15-tap box sum: PE banded-matrix matmuls in fp16 with the
    1/225 scale folded in as 1/256 (dc scaled by 225/256; w scaled by
    256/225 to compensate); all 4 row-groups accumulate into one
    [P,2048] PSUM tile copied to SBUF in a single ACT op
  - top-5% threshold: 2-level stripe-grid seed (L1 half-sampled, L2
    full) bracketing the threshold, then 2 secant steps with global
    counts (c0/c2 half-sampled slope, c(tau1) full). All counts on ACT
    as Sign(tau - dc) (scale=-1, bias=tau: no negation smalls);
    cross-partition reduction + broadcast via ones matmul
  - finals: exact recount at tau2 on ACT; masked channel sums as DVE
    stt (is_ge * img) with accum; A = sums/count; transmission: ACT
    Identity (scale=-w', bias=1.001) + DVE fast reciprocal + fp16
    min-clamp; dehaze d=(img-A)*rr on DVE, clip via min/addmax on DVE
    (ch0) or ACT Relu pairs out = Relu(1 - Relu((1-A) - d)) (ch1/ch2);
    one batched fp16 store per image, upcast to fp32 on host
"""
import os
import numpy as np

import concourse.bacc as bacc
import concourse.tile as tile
import concourse.mybir as mybir
from concourse.bass_utils import run_bass_kernel_spmd

F32 = mybir.dt.float32
F16 = mybir.dt.float16
ALU = mybir.AluOpType
ACTF = mybir.ActivationFunctionType

P = 128
H = W = 512
G = H // P              # 4 row-groups
NPC = 4                 # images per core
K = 13107               # int(512*512*0.05)
KF = float(K)
KH = KF / 2.0           # target in half-sample count units
NTOT = float(H * W)

# stripe-grid seed constants (dc' = dc*225/256 lives in [0,1))
D1 = 1.0 / 128.0            # level-1 grid step
L1_OFF = -3.0               # lo1 = (jc1 + L1_OFF) * D1
D2 = 5.0 * D1 / 128.0       # level-2 grid step
L2_OFF = -10.0              # lo2 = lo1 + (jc2 + L2_OFF) * D2
W2 = 17.0 * D2              # bracket width for the secant stage
# counts use S' = sum sign(tau - x); count_ge = (N - S')/2
THR1N = 1024.0 - KF / 128.0      # L1 stripe: S'_p <= THR1N
THR2N = 2048.0 - 2.0 * KF / 128.0  # L2 stripe
RRMAX = float(np.float32(1.0) / np.float32(0.101))
WSCALE = 256.0 / 225.0


def make_consts16() -> np.ndarray:
    k = np.arange(P)[:, None]
    m = np.arange(P)[None, :]
    v = np.float16(1.0 / 256.0)
    bdiag = (np.abs(k - m) <= 7).astype(np.float16) * v
    bup = ((k - m) >= 121).astype(np.float16) * v
    bdn = ((m - k) >= 121).astype(np.float16) * v
    return np.concatenate([bdiag, bup, bdn], axis=1)  # [128, 384] f16


def make_consts32() -> np.ndarray:
    return np.ones((P, P), dtype=np.float32)


def build(nc):
    img_in = nc.dram_tensor("img", [NPC, 3, H, W], F32, kind="ExternalInput").ap()
    w_in = nc.dram_tensor("w", [NPC], F32, kind="ExternalInput").ap()
    c16_in = nc.dram_tensor("c16", [P, 3 * P], F16, kind="ExternalInput").ap()
    c32_in = nc.dram_tensor("c32", [P, P], F32, kind="ExternalInput").ap()
    out_d = nc.dram_tensor("out", [NPC, 3, H, W], F16, kind="ExternalOutput").ap()

    V = nc.vector
    S = nc.scalar

    with tile.TileContext(nc) as tc:
        with (
            tc.tile_pool(name="const", bufs=1) as const_pool,
            tc.tile_pool(name="img32", bufs=2) as img32p,
            tc.tile_pool(name="img16", bufs=4) as img16p,
            tc.tile_pool(name="dcp", bufs=4) as dc_pool,
            tc.tile_pool(name="mnp", bufs=1) as mnp,
            tc.tile_pool(name="shp", bufs=2) as shp,
            tc.tile_pool(name="pbp", bufs=1) as pbp,
            tc.tile_pool(name="s16", bufs=1) as s16,
            tc.tile_pool(name="rrp", bufs=2) as rrp,
            tc.tile_pool(name="dp", bufs=2) as dpp,
            tc.tile_pool(name="tm32", bufs=1) as tm32p,
            tc.tile_pool(name="small", bufs=4) as small,
            tc.tile_pool(name="vband", bufs=1, space="PSUM") as vband,
            tc.tile_pool(name="cntps", bufs=2, space="PSUM") as cnt_ps,
            tc.tile_pool(name="totps", bufs=1, space="PSUM") as tot_psp,
            tc.tile_pool(name="miscps", bufs=1, space="PSUM") as misc_ps,
        ):
            c16 = const_pool.tile([P, 3 * P], F16)
            nc.sync.dma_start(c16[:], c16_in[:])
            bdiag = c16[:, 0:P]
            bup = c16[:, P:2 * P]
            bdn = c16[:, 2 * P:3 * P]
            ones = const_pool.tile([P, P], F32)
            nc.sync.dma_start(ones[:], c32_in[:])

            # iota grid: gridf[p] = p and g1[p] = p*D1
            grid_i = const_pool.tile([P, 1], mybir.dt.int32)
            nc.gpsimd.iota(grid_i[:], pattern=[[0, 1]], base=0,
                           channel_multiplier=1)
            gridf = const_pool.tile([P, 1], F32)
            V.tensor_copy(gridf[:], grid_i[:])
            g1 = const_pool.tile([P, 1], F32)
            V.tensor_scalar(out=g1[:], in0=gridf[:], scalar1=D1,
                            scalar2=None, op0=ALU.mult)

            # w broadcast to all partitions, scaled by -256/225
            w_sb = const_pool.tile([1, NPC], F32)
            nc.sync.dma_start(w_sb[:], w_in.rearrange("(p a) -> p a", p=1))
            w4_ps = misc_ps.tile([P, NPC], F32, tag="w4")
            nc.tensor.matmul(w4_ps[:], lhsT=ones[0:1, :], rhs=w_sb[:],
                             start=True, stop=True)
            negw4 = const_pool.tile([P, NPC], F32)
            V.tensor_scalar(out=negw4[:], in0=w4_ps[:], scalar1=-WSCALE,
                            scalar2=None, op0=ALU.mult)
            b1001 = const_pool.tile([P, 1], F32)
            V.memset(b1001[:], 1.001)
            bone = const_pool.tile([P, 1], F32)
            V.memset(bone[:], 1.0)
            bthr1 = const_pool.tile([P, 1], F32)
            V.memset(bthr1[:], THR1N)
            bthr2 = const_pool.tile([P, 1], F32)
            V.memset(bthr2[:], THR2N)

            def phase1(i):
                """batched load + fp16 cast + channel-min + box filter."""
                t32 = img32p.tile([P, 3, G, W], F32, tag="i32")
                t16 = img16p.tile([P, 3, G, W], F16, tag="i16")
                for c in range(3):
                    nc.sync.dma_start(
                        t32[:, c],
                        img_in[i, c].rearrange("(g p) x -> p g x", p=P))
                    S.activation(t16[:, c].rearrange("p g x -> p (g x)"),
                                 t32[:, c].rearrange("p g x -> p (g x)"),
                                 ACTF.Copy)

                mn = mnp.tile([P, G, W], F16, tag="mn")
                V.tensor_tensor(out=mn[:], in0=t16[:, 0], in1=t16[:, 1],
                                op=ALU.min)
                V.tensor_tensor(out=mn[:], in0=mn[:], in1=t16[:, 2],
                                op=ALU.min)

                Pb = pbp.tile([P, 2560], F32, tag="pb")
                V.memset(Pb[:, 0:1], 0.0)
                mn_flat = mn[:].rearrange("p g x -> p (g x)")
                V.tensor_tensor_scan(
                    out=Pb[:, 1:2049], data0=mn_flat, data1=mn_flat,
                    initial=0.0, op0=ALU.add, op1=ALU.bypass)
                sh = shp.tile([P, G, W], F16, tag="sh")
                pv = Pb[:, 1:2049].rearrange("p (g x) -> p g x", g=G)
                V.tensor_tensor(
                    out=sh[:, :, 8:505], in0=pv[:, :, 15:512],
                    in1=pv[:, :, 0:497], op=ALU.subtract)
                # group-strided views of Pb for the left/right edge columns
                pvL = Pb[:, 8:2056].rearrange("p (g x) -> p g x", x=W)
                pvB = Pb[:, 0:2048].rearrange("p (g x) -> p g x", x=W)
                V.tensor_tensor(
                    out=sh[:, :, 0:8], in0=pvL[:, :, 0:8],
                    in1=pvB[:, :, 0:1].to_broadcast([P, G, 8]),
                    op=ALU.subtract)
                pvR = Pb[:, 512:2560].rearrange("p (g x) -> p g x", x=W)
                pvS = Pb[:, 498:2546].rearrange("p (g x) -> p g x", x=W)
                V.tensor_tensor(
                    out=sh[:, :, 505:512],
                    in0=pvR[:, :, 0:1].to_broadcast([P, G, 7]),
                    in1=pvS[:, :, 0:7], op=ALU.subtract)

                # vertical band matmuls, all groups into one PSUM tile
                ps = vband.tile([P, G, W], F32, tag="vps")
                for gp in range(G):
                    mms = [(bdiag, gp)]
                    if gp > 0:
                        mms.append((bup, gp - 1))
                    if gp < G - 1:
                        mms.append((bdn, gp + 1))
                    for j, (band, gsrc) in enumerate(mms):
                        nc.tensor.matmul(ps[:, gp], lhsT=band,
                                         rhs=sh[:, gsrc, :],
                                         start=(j == 0), stop=(j == len(mms) - 1))
                dc = dc_pool.tile([P, G, W], F32, tag="dc")
                S.copy(dc[:].rearrange("p g x -> p (g x)"),
                       ps[:].rearrange("p g x -> p (g x)"))
                return t16, dc

            def halfview(dcf):
                return dcf.rearrange("p (n two) -> p n two", two=2)[:, :, 0]

            def count_matmul(part, n):
                """sum part [P,n] across partitions -> PSUM [P,n] bcast."""
                cps = cnt_ps.tile([P, n], F32, tag="cps")
                nc.tensor.matmul(cps[:], lhsT=ones, rhs=part[:, 0:n],
                                 start=True, stop=True)
                return cps

            def seed1(i, dc, st):
                """L1 stripe pass (ACT, half-sampled) -> lo1 tile."""
                dcf = dc[:].rearrange("p g x -> p (g x)")
                scr = s16.tile([P, 2048], F16, tag="sA")
                s1 = small.tile([P, 1], F32, tag="s1")
                S.activation(scr[:, 0:1024], halfview(dcf), ACTF.Sign,
                             bias=g1[:], scale=-1.0, accum_out=s1[:])
                # mk = sign(THR1N - S') on ACT; jc = (sum(mk)+128)/2
                mk = small.tile([P, 1], F32, tag="mk")
                S.activation(mk[:], s1[:], ACTF.Sign, bias=bthr1[:],
                             scale=-1.0)
                jc = count_matmul(mk, 1)
                lo1 = small.tile([P, 1], F32, tag=f"lo1_{i}")
                V.tensor_scalar(out=lo1[:], in0=jc[:], scalar1=0.5 * D1,
                                scalar2=(64.0 + L1_OFF) * D1, op0=ALU.mult,
                                op1=ALU.add)
                st["lo1"] = lo1

            def seed2(i, dc, st):
                """L2 stripe pass (ACT, full) -> lo2, t2v (bracket)."""
                dcf = dc[:].rearrange("p g x -> p (g x)")
                lo1 = st["lo1"]
                t2g = small.tile([P, 1], F32, tag="t2g")
                V.scalar_tensor_tensor(
                    out=t2g[:], in0=gridf[:], scalar=D2, in1=lo1[:],
                    op0=ALU.mult, op1=ALU.add)
                scr = s16.tile([P, 2048], F16, tag="sA")
                s2 = small.tile([P, 1], F32, tag="s2")
                S.activation(scr[:], dcf, ACTF.Sign, bias=t2g[:], scale=-1.0,
                             accum_out=s2[:])
                mk = small.tile([P, 1], F32, tag="mk")
                S.activation(mk[:], s2[:], ACTF.Sign, bias=bthr2[:],
                             scale=-1.0)
                jc2 = count_matmul(mk, 1)
                q = small.tile([P, 1], F32, tag="q")
                V.tensor_scalar(out=q[:], in0=jc2[:], scalar1=0.5 * D2,
                                scalar2=(64.0 + L2_OFF) * D2, op0=ALU.mult,
                                op1=ALU.add)
                lo2 = small.tile([P, 1], F32, tag=f"lo2_{i}")
                V.tensor_tensor(out=lo2[:], in0=q[:], in1=lo1[:], op=ALU.add)
                t2v = small.tile([P, 1], F32, tag=f"t2v_{i}")
                V.tensor_scalar(out=t2v[:], in0=lo2[:], scalar1=W2,
                                scalar2=None, op0=ALU.add)
                st["lo2"], st["t2v"] = lo2, t2v

            def slope(i, dc, st):
                """c0/c2 (ACT, half-sampled) -> rW, tau1 seed."""
                dcf = dc[:].rearrange("p g x -> p (g x)")
                dch = halfview(dcf)
                lo2, t2v = st["lo2"], st["t2v"]
                part2 = small.tile([P, 2], F32, tag=f"pc_{i}")
                scra = s16.tile([P, 2048], F16, tag="sA")
                S.activation(scra[:, 0:1024], dch, ACTF.Sign, bias=lo2[:],
                             scale=-1.0, accum_out=part2[:, 0:1])
                scrb = s16.tile([P, 2048], F16, tag="sA")
                S.activation(scrb[:, 0:1024], dch, ACTF.Sign, bias=t2v[:],
                             scale=-1.0, accum_out=part2[:, 1:2])
                cps = count_matmul(part2, 2)
                # half-count = -0.5*S' + 65536 for both slots
                cv = small.tile([P, 2], F32, tag="cv")
                V.tensor_scalar(out=cv[:], in0=cps[:], scalar1=-0.5,
                                scalar2=65536.0, op0=ALU.mult, op1=ALU.add)
                diff = small.tile([P, 1], F32, tag="diff")
                V.tensor_tensor(out=diff[:], in0=cv[:, 0:1], in1=cv[:, 1:2],
                                op=ALU.subtract)
                V.tensor_scalar(out=diff[:], in0=diff[:], scalar1=0.5,
                                scalar2=None, op0=ALU.max)
                rd = small.tile([P, 1], F32, tag="rd")
                V.reciprocal(out=rd[:], in_=diff[:])
                rW = small.tile([P, 1], F32, tag=f"rW_{i}")
                V.tensor_scalar(out=rW[:], in0=rd[:], scalar1=W2,
                                scalar2=None, op0=ALU.mult)
                e0 = small.tile([P, 1], F32, tag="e0")
                V.tensor_scalar(out=e0[:], in0=cv[:, 0:1], scalar1=-KH,
                                scalar2=None, op0=ALU.add)
                tau1 = small.tile([P, 1], F32, tag=f"tau1_{i}")
                V.scalar_tensor_tensor(
                    out=tau1[:], in0=e0[:], scalar=rW[:], in1=lo2[:],
                    op0=ALU.mult, op1=ALU.add)
                V.tensor_scalar(out=tau1[:], in0=tau1[:], scalar1=lo2[:],
                                scalar2=t2v[:], op0=ALU.max, op1=ALU.min)
                st["rW"], st["tau1"] = rW, tau1
                st["tau2"] = tau1  # single secant step; finals recount exactly

            def refine(i, dc, st):
                """count at tau1 (ACT, full) -> tau2 (final threshold)."""
                dcf = dc[:].rearrange("p g x -> p (g x)")
                tau1, rW = st["tau1"], st["rW"]
                lo2, t2v = st["lo2"], st["t2v"]
                scra = s16.tile([P, 2048], F16, tag="sA")
                pc1 = small.tile([P, 1], F32, tag="pc1")
                S.activation(scra[:], dcf, ACTF.Sign, bias=tau1[:],
                             scale=-1.0, accum_out=pc1[:])
                ct = count_matmul(pc1, 1)
                # half-count units: c1h - KH = -0.25*S' + (65536 - KH)
                e1 = small.tile([P, 1], F32, tag="e1")
                V.tensor_scalar(out=e1[:], in0=ct[:], scalar1=-0.25,
                                scalar2=65536.0 - KH, op0=ALU.mult,
                                op1=ALU.add)
                tau2 = small.tile([P, 1], F32, tag=f"tau2_{i}")
                V.scalar_tensor_tensor(
                    out=tau2[:], in0=e1[:], scalar=rW[:], in1=tau1[:],
                    op0=ALU.mult, op1=ALU.add)
                V.tensor_scalar(out=tau2[:], in0=tau2[:], scalar1=lo2[:],
                                scalar2=t2v[:], op0=ALU.max, op1=ALU.min)
                st["tau2"] = tau2

            def finals_a(i, t16, dc, st):
                """recount + masked sums -> A3, omA3; tm (ACT)."""
                dcf = dc[:].rearrange("p g x -> p (g x)")
                tau2 = st["tau2"]
                tm = tm32p.tile([P, 2048], F32, tag="tm")
                S.activation(tm[:], dcf, ACTF.Identity, bias=b1001[:],
                             scale=negw4[:, i:i + 1])
                part4 = small.tile([P, 4], F32, tag=f"p4_{i}")
                scra = s16.tile([P, 2048], F16, tag="sA")
                S.activation(scra[:], dcf, ACTF.Sign, bias=tau2[:],
                             scale=-1.0, accum_out=part4[:, 0:1])
                for c in range(3):
                    scr = s16.tile([P, 2048], F16, tag="sD")
                    V.scalar_tensor_tensor(
                        out=scr[:], in0=dcf, scalar=tau2[:],
                        in1=t16[:, c].rearrange("p g x -> p (g x)"),
                        op0=ALU.is_ge, op1=ALU.mult,
                        accum_out=part4[:, c + 1:c + 2])
                tot = tot_psp.tile([P, 4], F32, tag="tot")
                nc.tensor.matmul(tot[:], lhsT=ones, rhs=part4[:],
                                 start=True, stop=True)
                cnt = small.tile([P, 1], F32, tag="cnt")
                V.tensor_scalar(out=cnt[:], in0=tot[:, 0:1], scalar1=-0.5,
                                scalar2=131072.0, op0=ALU.mult, op1=ALU.add)
                rc = small.tile([P, 1], F32, tag="rc")
                V.reciprocal(out=rc[:], in_=cnt[:])
                A3 = small.tile([P, 3], F32, tag=f"A3_{i}")
                V.tensor_tensor(out=A3[:], in0=tot[:, 1:4],
                                in1=rc[:].to_broadcast([P, 3]), op=ALU.mult)
                omA3 = small.tile([P, 3], F32, tag=f"om_{i}")
                V.tensor_scalar(out=omA3[:], in0=A3[:], scalar1=-1.0,
                                scalar2=1.0, op0=ALU.mult, op1=ALU.add)
                st["A3"], st["omA3"], st["tm"] = A3, omA3, tm

            def finals_b(i, t16, dc, st):
                """rr + dehaze + batched store."""
                dcf = dc[:].rearrange("p g x -> p (g x)")
                A3, omA3, tm = st["A3"], st["omA3"], st["tm"]
                # rr = min(1/tm, 1/0.101) fp16; 1/tm overwrites dc
                V.reciprocal_approx_fast(out=dcf, in_=tm[:])
                rr = rrp.tile([P, 2048], F16, tag="rr")
                V.tensor_scalar(out=rr[:], in0=dcf, scalar1=RRMAX,
                                scalar2=None, op0=ALU.min)
                d = dpp.tile([P, 3, G, W], F16, tag="d")
                # ACT-pair channels first so DVE-route ch finishes last and
                # stores are not gated on late ACT work; the last image uses
                # the DVE route for every channel to shorten the kernel tail
                order = (0, 1, 2) if i == NPC - 1 else (1, 2, 0)
                for c in order:
                    img_flat = t16[:, c].rearrange("p g x -> p (g x)")
                    dv = d[:, c].rearrange("p g x -> p (g x)")
                    V.scalar_tensor_tensor(
                        out=dv, in0=img_flat, scalar=A3[:, c:c + 1],
                        in1=rr[:], op0=ALU.subtract, op1=ALU.mult)
                    if c == 0 or i == NPC - 1:
                        V.tensor_scalar(out=dv, in0=dv,
                                        scalar1=omA3[:, c:c + 1],
                                        scalar2=None, op0=ALU.min)
                        V.tensor_scalar(out=dv, in0=dv,
                                        scalar1=A3[:, c:c + 1], scalar2=0.0,
                                        op0=ALU.add, op1=ALU.max)
                    else:
                        # out = Relu(1 - Relu((1-A) - d)): both clips on ACT
                        u = s16.tile([P, 2048], F16, tag="sD")
                        S.activation(u[:], dv, ACTF.Relu,
                                     bias=omA3[:, c:c + 1], scale=-1.0)
                        S.activation(dv, u[:], ACTF.Relu,
                                     bias=bone[:], scale=-1.0)
                    nc.sync.dma_start(
                        out_d[i, c].rearrange("(g p) x -> p g x", p=P),
                        d[:, c])

            imgs, dcs = [], []
            for i in range(NPC):
                a, b = phase1(i)
                imgs.append(a)
                dcs.append(b)
            states = [dict() for _ in range(NPC)]
            for i in range(NPC):
                seed1(i, dcs[i], states[i])
            for i in range(NPC):
                seed2(i, dcs[i], states[i])
            for i in range(NPC):
                slope(i, dcs[i], states[i])
            # interleave: ACT-heavy finals_a(i+1) overlaps DVE-heavy
            # finals_b(i)
            finals_a(0, imgs[0], dcs[0], states[0])
            finals_a(1, imgs[1], dcs[1], states[1])
            finals_b(0, imgs[0], dcs[0], states[0])
            finals_a(2, imgs[2], dcs[2], states[2])
            finals_b(1, imgs[1], dcs[1], states[1])
            finals_a(3, imgs[3], dcs[3], states[3])
            finals_b(2, imgs[2], dcs[2], states[2])
            finals_b(3, imgs[3], dcs[3], states[3])
    nc.compile()
    return nc


NCORES = 8
CONSTS16 = make_consts16()
CONSTS32 = make_consts32()
LAST_RESULT = None
_NC_CACHE = None


def _get_nc():
    global _NC_CACHE
    if _NC_CACHE is None:
        nc = bacc.Bacc("TRN2", target_bir_lowering=False, debug=False)
        _NC_CACHE = build(nc)
    return _NC_CACHE


def kernel(img: np.ndarray, w: np.ndarray) -> np.ndarray:
    global LAST_RESULT
    img = np.ascontiguousarray(np.asarray(img, dtype=np.float32))
    w = np.ascontiguousarray(np.asarray(w, dtype=np.float32))
    nc = _get_nc()
    in_maps = [
        {"img": img[i * NPC:(i + 1) * NPC], "w": w[i * NPC:(i + 1) * NPC],
         "c16": CONSTS16, "c32": CONSTS32}
        for i in range(NCORES)
    ]
    trace = bool(int(os.environ.get("DEHAZE_TRACE", "0")))
    res = run_bass_kernel_spmd(nc, in_maps, list(range(NCORES)), trace=trace)
    LAST_RESULT = res
    return np.concatenate(
        [r["out"] for r in res.results], axis=0).astype(np.float32)
